# revision 1
# baseline (speedup 1.0000x reference)
"""Multi-head attention with KV cache, sharded over 8 NeuronCores by head.

Problem (hardcoded shapes):
  x       [4, 512, 1024]      hidden states (B, T, D)
  k_prev  [4, 16, 3584, 64]   KV cache (B, H, PAST, HD)
  v_prev  [4, 16, 3584, 64]
  Wq/Wk/Wv/Wo [1024, 1024]    projection weights (torch Linear: y = x @ W.T)

Sharding: 16 heads / 8 cores = 2 heads per core (data stays full along batch).
Each core computes q/k/v projections for its 2 heads (column-parallel),
full attention for its heads, and a column-parallel o_proj partial
[2048, 1024]; the host sums the 8 partials (the o_proj all-reduce).

Device algorithm per core (fp32 accumulate, float32r matmul operands --
the TF32-like single-pass PE mode, ~12-13 effective mantissa bits, 4x the
fp32 matmul rate; measured end-to-end rel err ~2e-4):
  - qT/kT_new/vT_new = W_slice @ x^T   via PE, contracting D (xT fed from host)
  - per (batch, head): scores^T[key, q] = k @ q^T (K=HD=64 on partitions),
    streamed in key-chunks of 128 grouped 3 per PSUM tile; the causal mask on
    the 4 newest chunks is accumulated by an extra identity @ mask matmul;
    exp on ScalarE (no max subtraction -- scores are O(1) by construction);
    out^T[hd, q] accumulated in PSUM via lhsT = [v | 1] so row 64 of the
    accumulator is the softmax denominator; divide, then o_proj.
  Phases are software-pipelined: batch-0 KV DMAs and attention overlap the
  projection matmuls, batch b+1 KV setup overlaps batch b, o_proj per batch.
"""

import numpy as np

import concourse.bass as bass
import concourse.mybir as mybir
import concourse.tile as tile
from concourse import bacc
from concourse.bass_utils import run_bass_kernel_spmd
from concourse.masks import make_identity

B, T, D = 4, 512, 1024
H, HD = 16, 64
PAST = 3584
L = PAST + T            # 4096 == MAX_CACHE, nothing is trimmed
SCALE = 1.0 / np.sqrt(HD).astype(np.float32)
NCORES = 8
HPC = H // NCORES       # heads per core = 2
TOK = B * T             # 2048
NCH = L // 128          # 32 key chunks per (b, h)
PCH = PAST // 128       # 28 chunks from the cache
FP32 = mybir.dt.float32
NEG = -1.0e30

_cache = {}

# float32r: 4-byte fp32 variant the PE consumes at full rate (~12-13 mantissa
# bits effective, measured) vs 4 cycles/row for fp32. All matmul operands are
# stored as fp32r; producers (DMA / DVE copy / ACT exp) write the rounded form.
FP32R = mybir.dt.float32r


def _build():
    nc = bacc.Bacc(None, target_bir_lowering=False)

    xT = nc.dram_tensor("xT", [D, TOK], FP32R, kind="ExternalInput")
    wqT = nc.dram_tensor("wqT", [D, 128], FP32R, kind="ExternalInput")
    wkT = nc.dram_tensor("wkT", [D, 128], FP32R, kind="ExternalInput")
    wvT = nc.dram_tensor("wvT", [D, 128], FP32R, kind="ExternalInput")
    woT = nc.dram_tensor("woT", [128, D], FP32R, kind="ExternalInput")
    kTp = nc.dram_tensor("kTp", [B, 128, PAST], FP32R, kind="ExternalInput")
    vp = nc.dram_tensor("vp", [B, 128, HPC, PCH, HD + 1], FP32R, kind="ExternalInput")
    out = nc.dram_tensor("out", [TOK, D], FP32, kind="ExternalOutput")

    Exp = mybir.ActivationFunctionType.Exp
    mult = mybir.AluOpType.mult
    add = mybir.AluOpType.add

    # key-chunk groups: scores psum tiles hold up to 3 chunks (3 PSUM banks)
    groups = [list(range(s, min(s + 3, NCH))) for s in range(0, NCH, 3)]

    with tile.TileContext(nc) as tc:
        with (
            tc.tile_pool(name="const", bufs=1) as const,
            tc.tile_pool(name="persist", bufs=1) as persist,
            tc.tile_pool(name="kv", bufs=2) as kv,
            tc.tile_pool(name="pt", bufs=2) as ptp,
            tc.tile_pool(name="div", bufs=2) as divp,
            tc.tile_pool(name="stage", bufs=1) as stage,
            tc.tile_pool(name="acc_ps", bufs=1, space="PSUM") as accp,
            tc.tile_pool(name="flex_ps", bufs=1, space="PSUM") as flexp,
        ):
            # ---- constants ----
            identity = const.tile([128, 128], FP32)
            make_identity(nc, identity)
            identity_r = const.tile([128, 128], FP32R)
            nc.vector.tensor_copy(identity_r, identity)
            masks = []
            for r in range(4):
                m = const.tile([128, T], FP32, tag=f"mask{r}")
                nc.gpsimd.memset(m, 0.0)
                # keep 0 where query i >= key-token (128r + kk), else NEG
                nc.gpsimd.affine_select(
                    out=m, in_=m, compare_op=mybir.AluOpType.is_ge,
                    fill=NEG, base=-128 * r, channel_multiplier=-1,
                    pattern=[[1, T]],
                )
                mr = const.tile([128, T], FP32R, tag=f"maskr{r}", name=f"maskr{r}")
                nc.vector.tensor_copy(mr, m)
                masks.append(mr)

            ones_c = const.tile([128, 1], FP32)
            nc.gpsimd.memset(ones_c, 1.0)
            warm = const.tile([1, 1], FP32)
            nc.scalar.activation(warm, ones_c[:1, :], Exp)
            ones_r = const.tile([1, HD], FP32R)
            nc.vector.tensor_copy(ones_r, ones_c[:1, :].to_broadcast([1, HD]))

            # ---- persistent SBUF ----
            woT_s = persist.tile([128, D], FP32R)
            qT = persist.tile([128, TOK], FP32R, tag="qT")
            kTn = persist.tile([128, TOK], FP32, tag="kTn")
            vTn = persist.tile([128, TOK], FP32, tag="vTn")
            oT = persist.tile([128, TOK], FP32R, tag="oT")

            def setup_batch(b, kT=None, va=None):
                bsl = bass.ts(b, T)
                if kT is None:
                    kT = kv.tile([128, L], FP32R, tag="kT", name=f"kT{b}")
                    nc.sync.dma_start(kT[:, :PAST], kTp[b, :, :])
                nc.vector.tensor_copy(kT[:, PAST:], kTn[:, bsl])
                if va is None:
                    va = kv.tile(
                        [128, HPC, NCH, HD + 1], FP32R, tag="va", name=f"va{b}"
                    )
                    nc.sync.dma_start(va[:, :, :PCH, :], vp[b, :, :, :, :])
                nc.vector.tensor_copy(
                    va[:, :, PCH:, HD],
                    ones_c[:, :, None].to_broadcast([128, HPC, NCH - PCH]),
                )
                for h in range(HPC):
                    hsl = slice(h * HD, (h + 1) * HD)
                    for tt in range(T // 128):
                        tp = flexp.tile([128, 512], FP32, tag="flex")
                        nc.tensor.transpose(
                            tp[:, :HD],
                            vTn[hsl, b * T + tt * 128 : b * T + (tt + 1) * 128],
                            identity[hsl, hsl],
                        )
                        nc.vector.tensor_copy(va[:, h, PCH + tt, :HD], tp[:, :HD])
                return kT, va

            # ---- phase A: projections (q/k/v for this core's 2 heads) ----
            nxt = None
            with (
                tc.tile_pool(name="xw", bufs=1) as xw,
                tc.tile_pool(name="xs", bufs=1) as xs,
            ):
                xT_r = xT.rearrange("(ko p) t -> p ko t", p=128)
                w_s = {}
                for name, w in (("q", wqT), ("k", wkT), ("v", wvT)):
                    w_s[name] = xw.tile(
                        [128, D // 128, 128], FP32R, tag=f"w{name}", name=f"w{name}"
                    )
                    if name == "q":
                        nc.sync.dma_start(
                            w_s[name], w.rearrange("(ko p) m -> p ko m", p=128)
                        )
                        xT_s0 = xs.tile([128, D // 128, 512], FP32R, tag="xT")
                        half = D // 256
                        nc.sync.dma_start(
                            xT_s0[:, :half, :], xT_r[:, :half, :512]
                        )
                        nc.sync.dma_start(
                            xT_s0[:, half:, :], xT_r[:, half:, :512]
                        )
                kT0 = kv.tile([128, L], FP32R, tag="kT", name="kT0")
                nc.sync.dma_start(kT0[:, : 12 * 128], kTp[0, :, : 12 * 128])
                va0 = kv.tile(
                    [128, HPC, NCH, HD + 1], FP32R, tag="va", name="va0"
                )
                nc.sync.dma_start(va0[:, :, :12, :], vp[0, :, :, :12, :])
                for name, w in (("k", wkT), ("v", wvT)):
                    nc.sync.dma_start(
                        w_s[name], w.rearrange("(ko p) m -> p ko m", p=128)
                    )
                nc.sync.dma_start(kT0[:, 12 * 128 : PAST], kTp[0, :, 12 * 128 :])
                nc.sync.dma_start(va0[:, :, 12:PCH, :], vp[0, :, :, 12:, :])
                def proj_tc(tcn, xT_s=None):
                    if xT_s is None:
                        xT_s = xs.tile(
                            [128, D // 128, 512], FP32R, tag="xT", name="xT_s"
                        )
                        half = D // 256
                        nc.sync.dma_start(
                            xT_s[:, :half, :], xT_r[:, :half, bass.ts(tcn, 512)]
                        )
                        nc.sync.dma_start(
                            xT_s[:, half:, :], xT_r[:, half:, bass.ts(tcn, 512)]
                        )
                    for name, dst in (("q", qT), ("k", kTn), ("v", vTn)):
                        ps = flexp.tile([128, 512], FP32, tag="flex")
                        for ko in range(D // 128):
                            nc.tensor.matmul(
                                ps,
                                lhsT=w_s[name][:, ko, :],
                                rhs=xT_s[:, ko, :],
                                start=(ko == 0),
                                stop=(ko == D // 128 - 1),
                            )
                        nc.vector.tensor_copy(dst[:, bass.ts(tcn, 512)], ps)

                proj_tc(0, xT_s=xT_s0)
                nxt = setup_batch(0, kT=kT0, va=va0)
                proj_tc(1)

                nc.sync.dma_start(woT_s, woT[:, :])

                # ---- phase B: attention per (batch, head) ----
                scp_cm = tc.tile_pool(name="sc_ps", bufs=2, space="PSUM")
                scp = scp_cm.__enter__()
                for b in range(B):
                    bsl = bass.ts(b, T)
                    kT, va = nxt
                    if b + 2 < B:
                        proj_tc(b + 2)
                    if b + 1 < B:
                        nxt = setup_batch(b + 1)

                    for h in range(HPC):
                        hsl = slice(h * HD, (h + 1) * HD)
                        acc = accp.tile([HD + 1, 512], FP32, tag="acc")
                        qTh = qT[hsl, bsl]
                        for g in groups:
                            ng = len(g)
                            ps = scp.tile([128, 3 * 512], FP32, tag="sc")
                            for j, cc in enumerate(g):
                                masked = cc >= PCH
                                # queries < off see nothing from chunk cc
                                off = max(0, (cc - PCH) * 128)
                                nc.tensor.matmul(
                                    ps[:, j * 512 + off : (j + 1) * 512],
                                    lhsT=kT[hsl, bass.ts(cc, 128)],
                                    rhs=qTh[:, off:],
                                    start=True,
                                    stop=not masked,
                                )
                                if masked:
                                    nc.tensor.matmul(
                                        ps[:, j * 512 + off : (j + 1) * 512],
                                        lhsT=identity_r,
                                        rhs=masks[cc - PCH][:, off:],
                                        start=False,
                                        stop=True,
                                        skip_group_check=True,
                                    )
                            pT = ptp.tile([128, 3 * 512], FP32R, tag="pT")
                            nc.scalar.activation(
                                pT[:, : ng * 512], ps[:, : ng * 512], Exp
                            )
                            for j, cc in enumerate(g):
                                off = max(0, (cc - PCH) * 128)
                                nc.tensor.matmul(
                                    acc[:, off:],
                                    lhsT=va[:, h, cc, :],
                                    rhs=pT[:, j * 512 + off : (j + 1) * 512],
                                    start=(cc == 0),
                                    stop=(cc == NCH - 1),
                                    skip_group_check=True,
                                )
                        # evict accumulator to SBUF at once (frees the PSUM
                        # bank for the next head); denominator in row 64
                        asb = divp.tile([HD + 1, 512], FP32, tag="asb")
                        nc.vector.tensor_copy(asb, acc)
                        if b == B - 1 and h == HPC - 1:
                            r0r = divp.tile([1, 512], FP32R, tag="r0r")
                            with nc.allow_low_precision(
                                reason="fp32r reciprocal feeds broadcast matmul"
                            ):
                                nc.vector.reciprocal(r0r, asb[HD : HD + 1, :])
                            bcp = flexp.tile(
                                [HD, 512], FP32, tag="flex", name="bcp"
                            )
                            nc.tensor.matmul(
                                bcp, lhsT=ones_r, rhs=r0r, start=True, stop=True
                            )
                            nc.vector.tensor_tensor(
                                oT[hsl, bsl], asb[:HD, :], bcp, mult
                            )
                        else:
                            r0 = divp.tile([1, 512], FP32, tag="r0")
                            nc.vector.reciprocal(r0, asb[HD : HD + 1, :])
                            bc = divp.tile([HD, 512], FP32, tag="bc")
                            nc.gpsimd.partition_broadcast(bc, r0)
                            nc.vector.tensor_tensor(
                                oT[hsl, bsl], asb[:HD, :], bc, mult
                            )

                    # ---- column-parallel o_proj for this batch ----
                    out_r = out[bsl, :].rearrange("(tt p) d -> p tt d", p=128)
                    if b == B - 1:
                        ostl = stage.tile(
                            [128, T // 128, D], FP32, tag="ost", name="ostl"
                        )
                        for tt in range(T // 128):
                            tsl = slice(b * T + tt * 128, b * T + (tt + 1) * 128)
                            for nh in range(2):
                                ps = scp.tile([128, 3 * 512], FP32, tag="sc")
                                ps = ps[:, :512]
                                nc.tensor.matmul(
                                    ps,
                                    lhsT=oT[:, tsl],
                                    rhs=woT_s[:, bass.ts(nh, 512)],
                                    start=True,
                                    stop=True,
                                )
                                if nh == 1:
                                    nc.scalar.copy(
                                        ostl[:, tt, bass.ts(nh, 512)], ps
                                    )
                                else:
                                    nc.vector.tensor_copy(
                                        ostl[:, tt, bass.ts(nh, 512)], ps
                                    )
                            nc.sync.dma_start(out_r[:, tt, :], ostl[:, tt, :])
                    else:
                        ost = stage.tile([128, T // 128, D], FP32, tag="ost")
                        for tt in range(T // 128):
                            tsl = slice(b * T + tt * 128, b * T + (tt + 1) * 128)
                            for nh in range(2):
                                ps = flexp.tile([128, 512], FP32, tag="flex")
                                nc.tensor.matmul(
                                    ps,
                                    lhsT=oT[:, tsl],
                                    rhs=woT_s[:, bass.ts(nh, 512)],
                                    start=True,
                                    stop=True,
                                )
                                nc.vector.tensor_copy(
                                    ost[:, tt, bass.ts(nh, 512)], ps
                                )
                            nc.sync.dma_start(out_r[:, tt, :], ost[:, tt, :])
                scp_cm.__exit__(None, None, None)

    nc.compile()
    return nc


def _pack_v(v):
    """[B, HPC, PAST, HD] -> [B, 128, HPC, PCH, HD+1] with ones in col HD."""
    out = np.empty((B, 128, HPC, PCH, HD + 1), np.float32)
    # v[b, h, c*128 + p, hd] -> out[b, p, h, c, hd]
    out[..., :HD] = v.reshape(B, HPC, PCH, 128, HD).transpose(0, 3, 1, 2, 4)
    out[..., HD] = 1.0
    return np.ascontiguousarray(out)


def _prep(x, k_prev, v_prev, Wq, Wk, Wv, Wo):
    """Host-side shard + layout marshalling (fp32, C-contiguous)."""
    f = np.float32
    x2 = np.ascontiguousarray(np.asarray(x, f).reshape(TOK, D))
    xT = np.ascontiguousarray(x2.T)
    k_prev = np.asarray(k_prev, f)
    v_prev = np.asarray(v_prev, f)
    Wq, Wk, Wv, Wo = (np.asarray(w, f) for w in (Wq, Wk, Wv, Wo))
    in_maps = []
    for c in range(NCORES):
        rows = slice(128 * c, 128 * (c + 1))
        hsl = slice(HPC * c, HPC * (c + 1))
        in_maps.append(
            {
                "xT": xT,
                "wqT": np.ascontiguousarray((Wq[rows, :] * SCALE).T),
                "wkT": np.ascontiguousarray(Wk[rows, :].T),
                "wvT": np.ascontiguousarray(Wv[rows, :].T),
                "woT": np.ascontiguousarray(Wo[:, rows].T),
                "kTp": np.ascontiguousarray(
                    k_prev[:, hsl, :, :].transpose(0, 1, 3, 2)
                ).reshape(B, 128, PAST),
                "vp": _pack_v(v_prev[:, hsl, :, :]),
            }
        )
    return in_maps


def kernel(x, k_prev, v_prev, Wq, Wk, Wv, Wo):
    if "nc" not in _cache:
        _cache["nc"] = _build()
    nc = _cache["nc"]
    in_maps = _prep(x, k_prev, v_prev, Wq, Wk, Wv, Wo)
    res = run_bass_kernel_spmd(nc, in_maps, core_ids=list(range(NCORES)))
    acc = np.zeros((TOK, D), np.float64)
    for r in res.results:
        acc += r["out"]
    return acc.astype(np.float32).reshape(B, T, D)



# revision 7
# speedup vs baseline: 1.2285x; 1.2285x over previous
"""Multi-head attention with KV cache, sharded over 8 NeuronCores by head.

Problem (hardcoded shapes):
  x       [4, 512, 1024]      hidden states (B, T, D)
  k_prev  [4, 16, 3584, 64]   KV cache (B, H, PAST, HD)
  v_prev  [4, 16, 3584, 64]
  Wq/Wk/Wv/Wo [1024, 1024]    projection weights (torch Linear: y = x @ W.T)

Sharding: 16 heads / 8 cores = 2 heads per core; each core computes its
2 heads' q/k/v projections, attention, and a column-parallel o_proj
partial [2048, 1024]; the host sums the 8 partials.

Device algorithm (all matmul operands bf16, fp32 PSUM accumulate;
measured end-to-end rel err ~1e-3 vs the 2e-2 gate):
  - q^T/k^T = W_slice @ x^T (contract D over 8 ko-steps); v computed
    directly in [token, channel] orientation so it lands in the
    [key-partition, head-dim] layout attention needs (no PE transposes).
  - scores^T[key, q] per 128-key chunk (K=HD=64 on partitions); causal
    mask on the 4 newest chunks added via identity @ mask matmul; exp on
    ScalarE (groups of 3 chunks = 3 PSUM banks, double buffered).
  - P.V in [query-partition, head-dim] orientation: out[q, hd] += over
    key chunks with lhsT = pT chunk [keys, q], rhs = [v | 1] [keys, 65]
    so column 64 accumulates the softmax denominator. 65-wide outputs
    cost 65 PE cycles/matmul vs 512 for the transposed orientation.
  - divide by denominator on DVE (free-dim broadcast), PE-transpose o to
    [channel, token] for o_proj, o_proj column-parallel, bf16 out.
  Scheduling: exp is the critical path (~123us on ScalarE). The P.V
  matmuls trail the score matmuls by one group so PE never blocks the
  exp stream, and projection/o_proj work is split into <=450ns "chores"
  threaded one-per-group between attention groups.
"""

import numpy as np
import ml_dtypes

import concourse.bass as bass
import concourse.mybir as mybir
import concourse.tile as tile
from concourse import bacc
from concourse.bass_utils import run_bass_kernel_spmd
from concourse.masks import make_identity

B, T, D = 4, 512, 1024
H, HD = 16, 64
PAST = 3584
L = PAST + T            # 4096 == MAX_CACHE, nothing is trimmed
SCALE = 1.0 / np.sqrt(HD).astype(np.float32)
NCORES = 8
HPC = H // NCORES       # heads per core = 2
TOK = B * T             # 2048
NCH = L // 128          # 32 key chunks per (b, h)
PCH = PAST // 128       # 28 chunks from the cache
QC = T // 128           # 4 query chunks of 128
KO = D // 128           # 8 contraction steps for projections
FP32 = mybir.dt.float32
BF16 = mybir.dt.bfloat16
BF = ml_dtypes.bfloat16
NEG = -1.0e30

_cache = {}


def _build():
    nc = bacc.Bacc(None, target_bir_lowering=False)

    xT = nc.dram_tensor("xT", [D, TOK], BF16, kind="ExternalInput")
    # weights pre-swizzled on host to [p, ko, m] so DMA rows are 2KB
    wq = nc.dram_tensor("wq", [128, KO, 128], BF16, kind="ExternalInput")
    wk = nc.dram_tensor("wk", [128, KO, 128], BF16, kind="ExternalInput")
    wv = nc.dram_tensor("wv", [128, KO, 128], BF16, kind="ExternalInput")
    woT = nc.dram_tensor("woT", [128, D], BF16, kind="ExternalInput")
    kTp = nc.dram_tensor("kTp", [B, 128, PAST], BF16, kind="ExternalInput")
    vp = nc.dram_tensor("vp", [B, 128, HPC, PCH, HD + 1], BF16, kind="ExternalInput")
    out = nc.dram_tensor("out", [TOK, D], BF16, kind="ExternalOutput")

    Exp = mybir.ActivationFunctionType.Exp
    mult = mybir.AluOpType.mult

    # key-chunk groups: a scores psum tile holds up to 3 chunks (3 banks)
    groups = [list(range(s, min(s + 3, NCH))) for s in range(0, NCH, 3)]
    xT_r = xT.rearrange("(ko p) t -> p ko t", p=128)

    with tile.TileContext(nc) as tc:
        with (
            tc.tile_pool(name="const", bufs=1) as const,
            tc.tile_pool(name="persist", bufs=1) as persist,
            tc.tile_pool(name="kv", bufs=2) as kv,
            tc.tile_pool(name="pt", bufs=3) as ptp,
            tc.tile_pool(name="div", bufs=2) as divp,
            tc.tile_pool(name="osb", bufs=2) as osbp,
            tc.tile_pool(name="stage", bufs=2) as stage,
            tc.tile_pool(name="xw", bufs=1) as xw,
            tc.tile_pool(name="xs", bufs=2) as xs,
            tc.tile_pool(name="acc_ps", bufs=1, space="PSUM") as accp,
            tc.tile_pool(name="flex_ps", bufs=1, space="PSUM") as flexp,
            tc.tile_pool(name="sc_ps", bufs=2, space="PSUM") as scp,
        ):
            # ---- DMAs first, ordered by first use ----
            w_s = {}
            for nm in ("q", "k", "v"):
                w_s[nm] = xw.tile([128, KO, 128], BF16, tag=f"w{nm}", name=f"w{nm}")
            xts, kts, vas = {}, {}, {}

            def load_x(bn):
                xt = xs.tile([128, KO, 512], BF16, tag="xT", name=f"x{bn}")
                half = KO // 2
                nc.sync.dma_start(xt[:, :half, :], xT_r[:, :half, bass.ts(bn, 512)])
                nc.sync.dma_start(xt[:, half:, :], xT_r[:, half:, bass.ts(bn, 512)])
                xts[bn] = xt

            def load_kv(bn):
                kT = kv.tile([128, L], BF16, tag="kT", name=f"kT{bn}")
                nc.sync.dma_start(kT[:, : 6 * 128], kTp[bn, :, : 6 * 128])
                nc.sync.dma_start(kT[:, 6 * 128 : PAST], kTp[bn, :, 6 * 128 :])
                va = kv.tile([128, HPC, NCH, HD + 1], BF16, tag="va", name=f"va{bn}")
                nc.sync.dma_start(va[:, :, :PCH, :], vp[bn, :, :, :, :])
                nc.gpsimd.memset(va[:, :, PCH:, HD], 1.0)
                kts[bn], vas[bn] = kT, va

            nc.sync.dma_start(w_s["q"], wq[:, :, :])
            load_x(0)
            nc.sync.dma_start(w_s["k"], wk[:, :, :])
            load_kv(0)
            nc.sync.dma_start(w_s["v"], wv[:, :, :])
            load_x(1)
            woT_s = persist.tile([128, D], BF16)
            nc.sync.dma_start(woT_s, woT[:, :])

            # ---- constants ----
            identity = const.tile([128, 128], FP32)
            make_identity(nc, identity)
            ident_b = const.tile([128, 128], BF16)
            nc.vector.tensor_copy(ident_b, identity)
            masks = []
            for r in range(4):
                m = const.tile([128, T], FP32, tag=f"mask{r}", name=f"m{r}")
                nc.gpsimd.memset(m, 0.0)
                # keep 0 where query t >= key-token (128r + p), else NEG
                nc.gpsimd.affine_select(
                    out=m, in_=m, compare_op=mybir.AluOpType.is_ge,
                    fill=NEG, base=-128 * r, channel_multiplier=-1,
                    pattern=[[1, T]],
                )
                mb_ = const.tile([128, T], BF16, tag=f"maskb{r}", name=f"mb{r}")
                nc.vector.tensor_copy(mb_, m)
                masks.append(mb_)
            ones_c = const.tile([128, 1], FP32)
            nc.gpsimd.memset(ones_c, 1.0)
            warm = const.tile([1, 1], FP32)
            nc.scalar.activation(warm, ones_c[:1, :], Exp)

            # ---- persistent SBUF ----
            qT = persist.tile([128, TOK], BF16, tag="qT")
            oT = persist.tile([128, B, QC, 128], BF16, tag="oT")

            # ---- projection chores: <= ~430ns of PE work each ----
            def chores_qk(bn, which):
                """4 chores of 2 ko-steps; evict on the last."""
                box = {}

                def mk(piece):
                    def go():
                        if piece == 0:
                            box["ps"] = flexp.tile(
                                [128, 512], FP32, tag="flex", name=f"p{which}{bn}"
                            )
                        ps = box["ps"]
                        for ko in (2 * piece, 2 * piece + 1):
                            nc.tensor.matmul(
                                ps, lhsT=w_s[which][:, ko, :], rhs=xts[bn][:, ko, :],
                                start=(ko == 0), stop=(ko == KO - 1),
                                skip_group_check=True,
                            )
                        if piece == 3:
                            if which == "q":
                                nc.vector.tensor_copy(qT[:, bass.ts(bn, 512)], ps)
                            else:
                                nc.vector.tensor_copy(kts[bn][:, PAST:], ps)
                    return go

                return [mk(p) for p in range(4)]

            def chores_v(bn):
                """4 chores, one 128-token chunk each (8 matmuls of 128)."""
                def mk(tt):
                    def go():
                        psv = flexp.tile(
                            [128, HPC, HD], FP32, tag="flex", name=f"pv{bn}{tt}"
                        )
                        for ko in range(KO):
                            nc.tensor.matmul(
                                psv, lhsT=xts[bn][:, ko, bass.ts(tt, 128)],
                                rhs=w_s["v"][:, ko, :],
                                start=(ko == 0), stop=(ko == KO - 1),
                                skip_group_check=True,
                            )
                        nc.vector.tensor_copy(vas[bn][:, :, PCH + tt, :HD], psv)
                    return go

                return [mk(tt) for tt in range(QC)]

            osts = {}

            def chores_oproj(bn):
                """8 chores: (tt, nh) matmul + evict; DMA out after nh==1."""
                out_chores = []
                for tt in range(QC):
                    for nh in range(2):
                        def go(tt=tt, nh=nh):
                            ps = flexp.tile(
                                [128, 512], FP32, tag="flex", name=f"po{bn}{tt}{nh}"
                            )
                            nc.tensor.matmul(
                                ps, lhsT=oT[:, bn, tt, :],
                                rhs=woT_s[:, bass.ts(nh, 512)],
                                start=True, stop=True,
                            )
                            if nh == 0:
                                osts[(bn, tt)] = stage.tile(
                                    [128, D], BF16, tag="ost", name=f"ost{bn}{tt}"
                                )
                            ost = osts[(bn, tt)]
                            nc.vector.tensor_copy(ost[:, bass.ts(nh, 512)], ps)
                            if nh == 1:
                                r0 = bn * T + tt * 128
                                nc.sync.dma_start(out[r0 : r0 + 128, :], ost)
                        out_chores.append(go)
                return out_chores

            # ---- attention pair: scores/exp run one group ahead of P.V ----
            def pair(b, h, osb, chores, ci):
                hsl = slice(h * HD, (h + 1) * HD)
                kT, va = kts[b], vas[b]
                acc = accp.tile(
                    [128, QC, HD + 1], FP32, tag="acc", name=f"acc{b}{h}"
                )
                pts = {}

                def pv(gi):
                    pT = pts[gi]
                    for j, cc in enumerate(groups[gi]):
                        for qc in range(max(0, cc - PCH), QC):
                            # one start/stop per psum bank per pair: start
                            # marks the WHOLE 2KB bank pending-zero, so each
                            # region's first write overwrites (self-zeroing)
                            nc.tensor.matmul(
                                acc[:, qc, :],
                                lhsT=pT[:, j, bass.ts(qc, 128)],
                                rhs=va[:, h, cc, :],
                                start=(cc == 0 and qc == 0),
                                stop=(cc == NCH - 1 and qc == QC - 1),
                                skip_group_check=True,
                            )

                for gi, g in enumerate(groups):
                    sc = scp.tile([128, 3, 512], FP32, tag="sc", name=f"sc{b}{h}{gi}")
                    for j, cc in enumerate(g):
                        r = cc - PCH
                        off = max(0, r * 128)
                        nc.tensor.matmul(
                            sc[:, j, off:],
                            lhsT=kT[hsl, bass.ts(cc, 128)],
                            rhs=qT[hsl, b * T + off : (b + 1) * T],
                            start=True, stop=r < 0,
                        )
                        if r >= 0:
                            nc.tensor.matmul(
                                sc[:, j, off:],
                                lhsT=ident_b, rhs=masks[r][:, off:],
                                start=False, stop=True, skip_group_check=True,
                            )
                    pT = ptp.tile([128, 3, 512], BF16, tag="pT", name=f"pT{b}{h}{gi}")
                    pts[gi] = pT
                    if len(g) == 3:
                        nc.scalar.activation(pT, sc, Exp)
                    else:  # last group: chunks 30 (q>=256) and 31 (q>=384)
                        nc.scalar.activation(pT[:, 0, 256:], sc[:, 0, 256:], Exp)
                        nc.scalar.activation(pT[:, 1, 384:], sc[:, 1, 384:], Exp)
                    if gi > 0:
                        pv(gi - 1)
                    if ci < len(chores):
                        chores[ci]()
                        ci += 1
                pv(len(groups) - 1)

                # divide by the softmax denominator (column 64 of acc)
                r_ = divp.tile([128, QC], FP32, tag="r", name=f"r{b}{h}")
                nc.vector.reciprocal(r_, acc[:, :, HD])
                nc.vector.tensor_tensor(
                    osb[:, :, h, :], acc[:, :, :HD],
                    r_[:, :, None].to_broadcast([128, QC, HD]), mult,
                )
                return ci

            def finish_batch(b, osb):
                # o [q, ch] -> oT [ch, q] per query chunk (both heads at once)
                tp = accp.tile([128, QC, 128], BF16, tag="acc", name=f"tp{b}")
                for qc in range(QC):
                    nc.tensor.transpose(tp[:, qc, :], osb[:, qc, :, :], ident_b)
                nc.vector.tensor_copy(oT[:, b], tp)

            # ---- prologue: only batch-0 q projection (ScalarE idle) ----
            ps_q = flexp.tile([128, 512], FP32, tag="flex", name="psq0")
            for ko in range(KO):
                nc.tensor.matmul(
                    ps_q, lhsT=w_s["q"][:, ko, :], rhs=xts[0][:, ko, :],
                    start=(ko == 0), stop=(ko == KO - 1), skip_group_check=True,
                )
            nc.vector.tensor_copy(qT[:, :512], ps_q)

            # ---- main loop ----
            for b in range(B):
                if b + 1 < B:
                    load_kv(b + 1)
                chores = []
                if b == 0:
                    chores += chores_qk(0, "k") + chores_v(0)
                else:
                    chores += chores_oproj(b - 1)
                if b + 1 < B:
                    chores += chores_qk(b + 1, "q") + chores_qk(b + 1, "k")
                    chores += chores_v(b + 1)
                osb = osbp.tile([128, QC, HPC, HD], BF16, tag="osb", name=f"osb{b}")
                ci = pair(b, 0, osb, chores, 0)
                if b + 2 < B:
                    load_x(b + 2)
                ci = pair(b, 1, osb, chores, ci)
                assert ci >= len(chores), (b, ci, len(chores))
                finish_batch(b, osb)

            # ---- tail: o_proj for the last batch (sc banks are free) ----
            bn = B - 1
            for tt in range(QC):
                ost = stage.tile([128, D], BF16, tag="ost", name=f"ostt{tt}")
                for nh in range(2):
                    ps = scp.tile([128, 3, 512], FP32, tag="sc", name=f"pt{tt}{nh}")
                    nc.tensor.matmul(
                        ps[:, 0, :], lhsT=oT[:, bn, tt, :],
                        rhs=woT_s[:, bass.ts(nh, 512)],
                        start=True, stop=True,
                    )
                    nc.vector.tensor_copy(ost[:, bass.ts(nh, 512)], ps[:, 0, :])
                r0 = bn * T + tt * 128
                nc.sync.dma_start(out[r0 : r0 + 128, :], ost)

    nc.compile()
    return nc


def _pack_v(v):
    """[B, HPC, PAST, HD] -> [B, 128, HPC, PCH, HD+1] bf16, ones in col HD."""
    o = np.empty((B, 128, HPC, PCH, HD + 1), np.float32)
    o[..., :HD] = v.reshape(B, HPC, PCH, 128, HD).transpose(0, 3, 1, 2, 4)
    o[..., HD] = 1.0
    return np.ascontiguousarray(o.astype(BF))


def _prep(x, k_prev, v_prev, Wq, Wk, Wv, Wo):
    """Host-side shard + layout marshalling (bf16, C-contiguous)."""
    f = np.float32
    x2 = np.asarray(x, f).reshape(TOK, D)
    xT = np.ascontiguousarray(x2.T.astype(BF))
    k_prev = np.asarray(k_prev, f)
    v_prev = np.asarray(v_prev, f)
    Wq, Wk, Wv, Wo = (np.asarray(w, f) for w in (Wq, Wk, Wv, Wo))

    def swz(wT):
        # [D, 128] -> [p, ko, m] with row (ko*128+p) -> [p, ko, :]
        return np.ascontiguousarray(
            wT.reshape(KO, 128, 128).transpose(1, 0, 2).astype(BF)
        )

    in_maps = []
    for c in range(NCORES):
        rows = slice(128 * c, 128 * (c + 1))
        hsl = slice(HPC * c, HPC * (c + 1))
        in_maps.append(
            {
                "xT": xT,
                "wq": swz((Wq[rows, :] * SCALE).T),
                "wk": swz(Wk[rows, :].T),
                "wv": swz(Wv[rows, :].T),
                "woT": np.ascontiguousarray(Wo[:, rows].T.astype(BF)),
                "kTp": np.ascontiguousarray(
                    k_prev[:, hsl, :, :].transpose(0, 1, 3, 2).astype(BF)
                ).reshape(B, 128, PAST),
                "vp": _pack_v(v_prev[:, hsl, :, :]),
            }
        )
    return in_maps


def kernel(x, k_prev, v_prev, Wq, Wk, Wv, Wo):
    if "nc" not in _cache:
        _cache["nc"] = _build()
    nc = _cache["nc"]
    in_maps = _prep(x, k_prev, v_prev, Wq, Wk, Wv, Wo)
    res = run_bass_kernel_spmd(nc, in_maps, core_ids=list(range(NCORES)))
    acc = np.zeros((TOK, D), np.float64)
    for r in res.results:
        acc += np.asarray(r["out"], np.float32)
    return acc.astype(np.float32).reshape(B, T, D)


# revision 32
# speedup vs baseline: 1.3004x; 1.0585x over previous
"""Multi-head attention with KV cache, sharded over 8 NeuronCores by head.

Problem (hardcoded shapes):
  x       [4, 512, 1024]      hidden states (B, T, D)
  k_prev  [4, 16, 3584, 64]   KV cache (B, H, PAST, HD)
  v_prev  [4, 16, 3584, 64]
  Wq/Wk/Wv/Wo [1024, 1024]    projection weights (torch Linear: y = x @ W.T)

Sharding: 16 heads / 8 cores = 2 heads per core; each core computes its
2 heads' q/k/v projections, attention, and a column-parallel o_proj
partial [2048, 1024]; the host sums the 8 partials.

Device algorithm (all matmul operands bf16, fp32 PSUM accumulate;
measured end-to-end rel err ~1e-3 vs the 2e-2 gate):
  - q^T/k^T = W_slice @ x^T (contract D over 8 ko-steps); v computed
    directly in [token, channel] orientation so it lands in the
    [key-partition, head-dim] layout attention needs (no PE transposes).
  - scores^T[key, q] per 128-key chunk (K=HD=64 on partitions); causal
    mask on the 4 newest chunks added via identity @ mask matmul; exp on
    ScalarE (groups of 3 chunks = 3 PSUM banks, double buffered).
  - P.V in [query-partition, head-dim] orientation: out[q, hd] += over
    key chunks with lhsT = pT chunk [keys, q], rhs = [v | 1] [keys, 65]
    so column 64 accumulates the softmax denominator. 65-wide outputs
    cost 65 PE cycles/matmul vs 512 for the transposed orientation.
  - divide by denominator on DVE (free-dim broadcast), PE-transpose o to
    [channel, token] for o_proj, o_proj column-parallel, bf16 out.
  Scheduling: exp is the critical path (~123us on ScalarE). The P.V
  matmuls trail the score matmuls by one group so PE never blocks the
  exp stream, and projection/o_proj work is split into <=450ns "chores"
  threaded one-per-group between attention groups.
"""

import numpy as np
import ml_dtypes

import concourse.bass as bass
import concourse.mybir as mybir
import concourse.tile as tile
from concourse import bacc
from concourse.bass_utils import run_bass_kernel_spmd
from concourse.masks import make_identity

B, T, D = 4, 512, 1024
H, HD = 16, 64
PAST = 3584
L = PAST + T            # 4096 == MAX_CACHE, nothing is trimmed
SCALE = 1.0 / np.sqrt(HD).astype(np.float32)
NCORES = 8
HPC = H // NCORES       # heads per core = 2
TOK = B * T             # 2048
NCH = L // 128          # 32 key chunks per (b, h)
PCH = PAST // 128       # 28 chunks from the cache
QC = T // 128           # 4 query chunks of 128
KO = D // 128           # 8 contraction steps for projections
FP32 = mybir.dt.float32
BF16 = mybir.dt.bfloat16
BF = ml_dtypes.bfloat16
NEG = -1.0e30

_cache = {}


def _build():
    nc = bacc.Bacc(None, target_bir_lowering=False)

    xT = nc.dram_tensor("xT", [D, TOK], BF16, kind="ExternalInput")
    # weights pre-swizzled on host to [p, ko, m] so DMA rows are 2KB
    wq = nc.dram_tensor("wq", [128, KO, 128], BF16, kind="ExternalInput")
    wk = nc.dram_tensor("wk", [128, KO, 128], BF16, kind="ExternalInput")
    wv = nc.dram_tensor("wv", [128, KO, 128], BF16, kind="ExternalInput")
    woT = nc.dram_tensor("woT", [128, D], BF16, kind="ExternalInput")
    kTp = nc.dram_tensor("kTp", [B, 128, PAST], BF16, kind="ExternalInput")
    vp = nc.dram_tensor("vp", [B, 128, HPC, PCH, HD + 1], BF16, kind="ExternalInput")
    out = nc.dram_tensor("out", [TOK, D], BF16, kind="ExternalOutput")

    Exp = mybir.ActivationFunctionType.Exp
    mult = mybir.AluOpType.mult

    # key-chunk groups: a scores psum tile holds up to 3 chunks (3 banks)
    groups = [list(range(s, min(s + 3, NCH))) for s in range(0, NCH, 3)]
    xT_r = xT.rearrange("(ko p) t -> p ko t", p=128)

    with tile.TileContext(nc) as tc:
        with (
            tc.tile_pool(name="const", bufs=1) as const,
            tc.tile_pool(name="persist", bufs=1) as persist,
            tc.tile_pool(name="kv", bufs=2) as kv,
            tc.tile_pool(name="pt", bufs=3) as ptp,
            tc.tile_pool(name="div", bufs=2) as divp,
            tc.tile_pool(name="osb", bufs=2) as osbp,
            tc.tile_pool(name="stage", bufs=4) as stage,
            tc.tile_pool(name="xw", bufs=1) as xw,
            tc.tile_pool(name="xs", bufs=2) as xs,
            tc.tile_pool(name="acc_ps", bufs=1, space="PSUM") as accp,
            tc.tile_pool(name="flex_ps", bufs=1, space="PSUM") as flexp,
            tc.tile_pool(name="sc_ps", bufs=2, space="PSUM") as scp,
        ):
            # ---- PE p-state warm-up: keep PE busy from t~0 so the clock
            # ramps to full speed before the first projection; operands are
            # uninitialized (outputs land in a psum bank nobody reads) ----
            dmy = const.tile([128, 128], BF16, tag="dmy")
            nc.gpsimd.memset(dmy, 0.0)
            dps = flexp.tile([128, 512], FP32, tag="flex", name="dps")
            for i in range(40):
                nc.tensor.matmul(
                    dps[:, :128], lhsT=dmy, rhs=dmy,
                    start=True, stop=True, skip_group_check=True,
                )

            # ---- DMAs next, ordered by first use ----
            w_s = {}
            for nm in ("q", "k", "v"):
                w_s[nm] = xw.tile([128, KO, 128], BF16, tag=f"w{nm}", name=f"w{nm}")
            xts, kts, vas = {}, {}, {}

            def load_x(bn, pieces=(4, 4)):
                xt = xs.tile([128, KO, 512], BF16, tag="xT", name=f"x{bn}")
                p0 = 0
                for np_ in pieces:
                    nc.sync.dma_start(
                        xt[:, p0 : p0 + np_, :],
                        xT_r[:, p0 : p0 + np_, bass.ts(bn, 512)],
                    )
                    p0 += np_
                xts[bn] = xt

            def load_kv(bn):
                kT = kv.tile([128, L], BF16, tag="kT", name=f"kT{bn}")
                nc.sync.dma_start(kT[:, : 6 * 128], kTp[bn, :, : 6 * 128])
                nc.sync.dma_start(kT[:, 6 * 128 : PAST], kTp[bn, :, 6 * 128 :])
                va = kv.tile([128, HPC, NCH, HD + 1], BF16, tag="va", name=f"va{bn}")
                nc.sync.dma_start(va[:, :, :PCH, :], vp[bn, :, :, :, :])
                nc.gpsimd.memset(va[:, :, PCH:, HD], 1.0)
                kts[bn], vas[bn] = kT, va

            nc.sync.dma_start(w_s["q"], wq[:, :, :])
            load_x(0, pieces=(4, 2, 1, 1))
            nc.sync.dma_start(w_s["k"], wk[:, :, :])
            load_kv(0)
            nc.sync.dma_start(w_s["v"], wv[:, :, :])
            load_x(1)
            woT_s = persist.tile([128, D], BF16)
            nc.sync.dma_start(woT_s, woT[:, :])

            # ---- constants ----
            identity = const.tile([128, 128], FP32)
            make_identity(nc, identity)
            ident_b = const.tile([128, 128], BF16)
            nc.vector.tensor_copy(ident_b, identity)
            masks = []
            for r in range(4):
                m = const.tile([128, T], FP32, tag=f"mask{r}", name=f"m{r}")
                nc.gpsimd.memset(m, 0.0)
                # keep 0 where query t >= key-token (128r + p), else NEG
                nc.gpsimd.affine_select(
                    out=m, in_=m, compare_op=mybir.AluOpType.is_ge,
                    fill=NEG, base=-128 * r, channel_multiplier=-1,
                    pattern=[[1, T]],
                )
                mb_ = const.tile([128, T], BF16, tag=f"maskb{r}", name=f"mb{r}")
                nc.vector.tensor_copy(mb_, m)
                masks.append(mb_)
            ones_c = const.tile([128, 1], FP32)
            nc.gpsimd.memset(ones_c, 1.0)
            warm = const.tile([1, 1], FP32)
            nc.scalar.activation(warm, ones_c[:1, :], Exp)

            # ---- persistent SBUF ----
            qT = persist.tile([128, TOK], BF16, tag="qT")
            oT = persist.tile([128, B, QC, 128], BF16, tag="oT")

            # ---- projection chores: <= ~430ns of PE work each ----
            def chores_qk(bn, which):
                """4 chores of 2 ko-steps; evict on the last."""
                box = {}

                def mk(piece):
                    def go():
                        if piece == 0:
                            box["ps"] = flexp.tile(
                                [128, 512], FP32, tag="flex", name=f"p{which}{bn}"
                            )
                        ps = box["ps"]
                        for ko in (2 * piece, 2 * piece + 1):
                            nc.tensor.matmul(
                                ps, lhsT=w_s[which][:, ko, :], rhs=xts[bn][:, ko, :],
                                start=(ko == 0), stop=(ko == KO - 1),
                                skip_group_check=True,
                            )
                        if piece == 3:
                            if which == "q":
                                nc.vector.tensor_copy(qT[:, bass.ts(bn, 512)], ps)
                            else:
                                nc.vector.tensor_copy(kts[bn][:, PAST:], ps)
                    return go

                return [mk(p) for p in range(4)]

            def chores_v(bn):
                """4 chores, one 128-token chunk each (8 matmuls of 128)."""
                def mk(tt):
                    def go():
                        psv = flexp.tile(
                            [128, HPC, HD], FP32, tag="flex", name=f"pv{bn}{tt}"
                        )
                        for ko in range(KO):
                            nc.tensor.matmul(
                                psv, lhsT=xts[bn][:, ko, bass.ts(tt, 128)],
                                rhs=w_s["v"][:, ko, :],
                                start=(ko == 0), stop=(ko == KO - 1),
                                skip_group_check=True,
                            )
                        nc.vector.tensor_copy(vas[bn][:, :, PCH + tt, :HD], psv)
                    return go

                return [mk(tt) for tt in range(QC)]

            osts = {}

            def chores_oproj(bn):
                """8 chores: (tt, nh) matmul + evict; DMA out after nh==1."""
                out_chores = []
                for tt in range(QC):
                    for nh in range(2):
                        def go(tt=tt, nh=nh):
                            ps = flexp.tile(
                                [128, 512], FP32, tag="flex", name=f"po{bn}{tt}{nh}"
                            )
                            nc.tensor.matmul(
                                ps, lhsT=oT[:, bn, tt, :],
                                rhs=woT_s[:, bass.ts(nh, 512)],
                                start=True, stop=True,
                            )
                            if nh == 0:
                                osts[(bn, tt)] = stage.tile(
                                    [128, D], BF16, tag="ost", name=f"ost{bn}{tt}"
                                )
                            ost = osts[(bn, tt)]
                            nc.vector.tensor_copy(ost[:, bass.ts(nh, 512)], ps)
                            if nh == 1:
                                r0 = bn * T + tt * 128
                                nc.sync.dma_start(out[r0 : r0 + 128, :], ost)
                        out_chores.append(go)
                return out_chores

            # one group of score matmuls (+ causal mask) and its exp
            def emit_group(bb, hh, gi):
                hsl = slice(hh * HD, (hh + 1) * HD)
                kT = kts[bb]
                g = groups[gi]
                sc = scp.tile([128, 3, 512], FP32, tag="sc", name=f"sc{bb}{hh}{gi}")
                for j, cc in enumerate(g):
                    r = cc - PCH
                    off = max(0, r * 128)
                    nc.tensor.matmul(
                        sc[:, j, off:],
                        lhsT=kT[hsl, bass.ts(cc, 128)],
                        rhs=qT[hsl, bb * T + off : (bb + 1) * T],
                        start=True, stop=r < 0,
                    )
                    if r >= 0:
                        nc.tensor.matmul(
                            sc[:, j, off:],
                            lhsT=ident_b, rhs=masks[r][:, off:],
                            start=False, stop=True, skip_group_check=True,
                        )
                pT = ptp.tile([128, 3, 512], BF16, tag="pT", name=f"pT{bb}{hh}{gi}")
                if len(g) == 3:
                    nc.scalar.activation(pT, sc, Exp)
                else:  # last group: chunks 30 (q>=256) and 31 (q>=384)
                    nc.scalar.activation(pT[:, 0, 256:], sc[:, 0, 256:], Exp)
                    nc.scalar.activation(pT[:, 1, 384:], sc[:, 1, 384:], Exp)
                return pT

            # ---- attention pair: scores/exp run one group ahead of P.V;
            # the NEXT pair's first group is prefetched before this pair's
            # trailing work so the exp stream never gaps at pair boundaries
            def pair(b, h, osb, chores, ci, last=False, pre=None, nxt=None):
                va = vas[b]
                nxt_pre = None
                acc = accp.tile(
                    [128, QC, HD + 1], FP32, tag="acc", name=f"acc{b}{h}"
                )
                pts = {}

                def finish_qcs(b, h, osb, qlo, qhi):
                    """Last pair only: divide + transpose + o_proj + DMA for
                    query chunks [qlo, qhi) as soon as they stop accumulating.
                    The two chains run in parallel across PE/ScalarE/DVE
                    (the exp stream is done or nearly done here)."""
                    nqc = qhi - qlo
                    r_ = divp.tile([128, nqc], FP32, tag="r", name=f"rL{qlo}")
                    nc.vector.reciprocal(r_, acc[:, qlo:qhi, HD])
                    nc.vector.tensor_tensor(
                        osb[:, qlo:qhi, h, :], acc[:, qlo:qhi, :HD],
                        r_[:, :, None].to_broadcast([128, nqc, HD]), mult,
                    )
                    tps = {}
                    for qc in range(qlo, qhi):
                        # the final call's second transpose goes to the (then
                        # dead) acc bank so the chains don't serialize on the
                        # flex slot; earlier calls must NOT touch the acc bank
                        # (a start=True there wipes the open accumulations)
                        pool, tag = ((accp, "acc") if qc > qlo and qlo >= 2
                                     else (flexp, "flex"))
                        tp = pool.tile([128, 128], BF16, tag=tag,
                                       name=f"tpL{qc}")
                        nc.tensor.transpose(tp, osb[:, qc, :, :], ident_b)
                        tps[qc] = tp
                    for qc in range(qlo, qhi):
                        if qc == qlo:
                            nc.vector.tensor_copy(oT[:, b, qc, :], tps[qc])
                        else:
                            nc.scalar.copy(oT[:, b, qc, :], tps[qc])
                    # 4 o_proj matmuls into 4 DISTINCT psum banks (scA, scB,
                    # then the flex/acc banks that held the dead transposes)
                    # so no matmul WARs on an eviction
                    pss = {}
                    osts = {}
                    for qc in range(qlo, qhi):
                        osts[qc] = stage.tile([128, D], BF16, tag="ost",
                                              name=f"osL{qc}")
                    for nh in range(2):
                        for i, qc in enumerate(range(qlo, qhi)):
                            if nh == 0:
                                ps = scp.tile([128, 3, 512], FP32, tag="sc",
                                              name=f"pL{qc}{nh}")[:, 0, :]
                            else:
                                pool, tag = ((flexp, "flex") if i == 0
                                             else (accp, "acc"))
                                if qlo < 2 and i > 0:
                                    pool, tag = scp, "sc"
                                ps = pool.tile([128, 512], FP32, tag=tag,
                                               name=f"pL{qc}{nh}")
                                if tag == "sc":
                                    ps = ps[:, :512]
                            nc.tensor.matmul(
                                ps, lhsT=oT[:, b, qc, :],
                                rhs=woT_s[:, bass.ts(nh, 512)],
                                start=True, stop=True,
                            )
                            pss[(qc, nh)] = ps
                    # evicts: the early pair leans on ScalarE (its exp stream
                    # is ending), the last pair leans on DVE
                    for i, (qc, nh) in enumerate(
                        (q, n) for n in range(2) for q in range(qlo, qhi)
                    ):
                        ev = osts[qc][:, bass.ts(nh, 512)]
                        act_side = (i != 3) if qlo < 2 else (i >= 2)
                        if act_side:
                            nc.scalar.copy(ev, pss[(qc, nh)])
                        else:
                            nc.vector.tensor_copy(ev, pss[(qc, nh)])
                    for qc in range(qlo, qhi):
                        r0 = b * T + qc * 128
                        if qlo >= 2 and qc == qhi - 1:
                            # last row block: split per half so the first
                            # half's transfer overlaps the second's eviction
                            for nh in range(2):
                                nc.sync.dma_start(
                                    out[r0 : r0 + 128, bass.ts(nh, 512)],
                                    osts[qc][:, bass.ts(nh, 512)],
                                )
                        else:
                            nc.sync.dma_start(out[r0 : r0 + 128, :], osts[qc])

                def pv(gi):
                    pT = pts[gi]
                    for j, cc in enumerate(groups[gi]):
                        for qc in range(max(0, cc - PCH), QC):
                            # one start/stop per psum bank per pair: start
                            # marks the WHOLE 2KB bank pending-zero, so each
                            # region's first write overwrites (self-zeroing)
                            nc.tensor.matmul(
                                acc[:, qc, :],
                                lhsT=pT[:, j, bass.ts(qc, 128)],
                                rhs=va[:, h, cc, :],
                                start=(cc == 0 and qc == 0),
                                stop=(cc == NCH - 1 and qc == QC - 1),
                                skip_group_check=True,
                            )

                npre = len(pre) if pre is not None else 0
                for gi in range(len(groups)):
                    if gi < npre:
                        pts[gi] = pre[gi]
                    else:
                        pts[gi] = emit_group(b, h, gi)
                    if gi == len(groups) - 1 and nxt is not None:
                        nxt_pre = (emit_group(nxt[0], nxt[1], 0),)
                    if gi > 0:
                        pv(gi - 1)
                    if ci < len(chores) and gi >= npre:
                        chores[ci]()
                        ci += 1
                    if last and gi == len(groups) - 1:
                        # qc 0/1 stopped at chunks 28/29 (group 9): finish
                        # them while ScalarE still runs the last exps
                        finish_qcs(b, h, osb, 0, 2)
                pv(len(groups) - 1)

                if last:
                    finish_qcs(b, h, osb, 2, 4)
                else:
                    # divide by the softmax denominator (column 64 of acc)
                    r_ = divp.tile([128, QC], FP32, tag="r", name=f"r{b}{h}")
                    nc.vector.reciprocal(r_, acc[:, :, HD])
                    nc.vector.tensor_tensor(
                        osb[:, :, h, :], acc[:, :, :HD],
                        r_[:, :, None].to_broadcast([128, QC, HD]), mult,
                    )
                return ci, nxt_pre

            def finish_batch(b, osb):
                # o [q, ch] -> oT [ch, q] per query chunk (both heads at once)
                tp = accp.tile([128, QC, 128], BF16, tag="acc", name=f"tp{b}")
                for qc in range(QC):
                    nc.tensor.transpose(tp[:, qc, :], osb[:, qc, :, :], ident_b)
                nc.vector.tensor_copy(oT[:, b], tp)

            # ---- prologue: only batch-0 q projection (ScalarE idle) ----
            ps_q = flexp.tile([128, 512], FP32, tag="flex", name="psq0")
            for ko in range(KO):
                nc.tensor.matmul(
                    ps_q, lhsT=w_s["q"][:, ko, :], rhs=xts[0][:, ko, :],
                    start=(ko == 0), stop=(ko == KO - 1), skip_group_check=True,
                )
            nc.vector.tensor_copy(qT[:, :512], ps_q)

            # ---- main loop ----
            pre = None
            for b in range(B):
                if b + 1 < B:
                    load_kv(b + 1)
                chores = []
                if b == 0:
                    chores += chores_qk(0, "k") + chores_v(0)
                else:
                    chores += chores_oproj(b - 1)
                if b + 1 < B:
                    chores += chores_qk(b + 1, "q") + chores_qk(b + 1, "k")
                    chores += chores_v(b + 1)
                osb = osbp.tile([128, QC, HPC, HD], BF16, tag="osb", name=f"osb{b}")
                ci, pre = pair(b, 0, osb, chores, 0, pre=pre, nxt=(b, 1))
                if b + 2 < B:
                    load_x(b + 2)
                nxt = (b + 1, 0) if b + 1 < B else None
                ci, pre = pair(
                    b, 1, osb, chores, ci, last=(b == B - 1), pre=pre, nxt=nxt
                )
                assert ci >= len(chores), (b, ci, len(chores))
                if b < B - 1:
                    finish_batch(b, osb)

    nc.compile()
    return nc


def _pack_v(v):
    """[B, HPC, PAST, HD] -> [B, 128, HPC, PCH, HD+1] bf16, ones in col HD."""
    o = np.empty((B, 128, HPC, PCH, HD + 1), np.float32)
    o[..., :HD] = v.reshape(B, HPC, PCH, 128, HD).transpose(0, 3, 1, 2, 4)
    o[..., HD] = 1.0
    return np.ascontiguousarray(o.astype(BF))


def _prep(x, k_prev, v_prev, Wq, Wk, Wv, Wo):
    """Host-side shard + layout marshalling (bf16, C-contiguous)."""
    f = np.float32
    x2 = np.asarray(x, f).reshape(TOK, D)
    xT = np.ascontiguousarray(x2.T.astype(BF))
    k_prev = np.asarray(k_prev, f)
    v_prev = np.asarray(v_prev, f)
    Wq, Wk, Wv, Wo = (np.asarray(w, f) for w in (Wq, Wk, Wv, Wo))

    def swz(wT):
        # [D, 128] -> [p, ko, m] with row (ko*128+p) -> [p, ko, :]
        return np.ascontiguousarray(
            wT.reshape(KO, 128, 128).transpose(1, 0, 2).astype(BF)
        )

    in_maps = []
    for c in range(NCORES):
        rows = slice(128 * c, 128 * (c + 1))
        hsl = slice(HPC * c, HPC * (c + 1))
        in_maps.append(
            {
                "xT": xT,
                "wq": swz((Wq[rows, :] * SCALE).T),
                "wk": swz(Wk[rows, :].T),
                "wv": swz(Wv[rows, :].T),
                "woT": np.ascontiguousarray(Wo[:, rows].T.astype(BF)),
                "kTp": np.ascontiguousarray(
                    k_prev[:, hsl, :, :].transpose(0, 1, 3, 2).astype(BF)
                ).reshape(B, 128, PAST),
                "vp": _pack_v(v_prev[:, hsl, :, :]),
            }
        )
    return in_maps


def kernel(x, k_prev, v_prev, Wq, Wk, Wv, Wo):
    if "nc" not in _cache:
        _cache["nc"] = _build()
    nc = _cache["nc"]
    in_maps = _prep(x, k_prev, v_prev, Wq, Wk, Wv, Wo)
    res = run_bass_kernel_spmd(nc, in_maps, core_ids=list(range(NCORES)))
    acc = np.zeros((TOK, D), np.float64)
    for r in res.results:
        acc += np.asarray(r["out"], np.float32)
    return acc.astype(np.float32).reshape(B, T, D)


# revision 38
# speedup vs baseline: 1.3080x; 1.0058x over previous
"""Multi-head attention with KV cache, sharded over 8 NeuronCores by head.

Problem (hardcoded shapes):
  x       [4, 512, 1024]      hidden states (B, T, D)
  k_prev  [4, 16, 3584, 64]   KV cache (B, H, PAST, HD)
  v_prev  [4, 16, 3584, 64]
  Wq/Wk/Wv/Wo [1024, 1024]    projection weights (torch Linear: y = x @ W.T)

Sharding: 16 heads / 8 cores = 2 heads per core; each core computes its
2 heads' q/k/v projections, attention, and a column-parallel o_proj
partial [2048, 1024]; the host sums the 8 partials.

Device algorithm (all matmul operands bf16, fp32 PSUM accumulate;
measured end-to-end rel err ~1e-3 vs the 2e-2 gate):
  - q^T/k^T = W_slice @ x^T (contract D over 8 ko-steps); v computed
    directly in [token, channel] orientation so it lands in the
    [key-partition, head-dim] layout attention needs (no PE transposes).
  - scores^T[key, q] per 128-key chunk (K=HD=64 on partitions); causal
    mask on the 4 newest chunks added via identity @ mask matmul; exp on
    ScalarE (groups of 3 chunks = 3 PSUM banks, double buffered).
  - P.V in [query-partition, head-dim] orientation: out[q, hd] += over
    key chunks with lhsT = pT chunk [keys, q], rhs = [v | 1] [keys, 65]
    so column 64 accumulates the softmax denominator. 65-wide outputs
    cost 65 PE cycles/matmul vs 512 for the transposed orientation.
  - divide by denominator on DVE (free-dim broadcast), PE-transpose o to
    [channel, token] for o_proj, o_proj column-parallel, bf16 out.
  Scheduling: exp is the critical path (~123us on ScalarE). The P.V
  matmuls trail the score matmuls by one group so PE never blocks the
  exp stream, and projection/o_proj work is split into <=450ns "chores"
  threaded one-per-group between attention groups.
"""

import numpy as np
import ml_dtypes

import concourse.bass as bass
import concourse.mybir as mybir
import concourse.tile as tile
from concourse import bacc
from concourse.bass_utils import run_bass_kernel_spmd
from concourse.masks import make_identity

B, T, D = 4, 512, 1024
H, HD = 16, 64
PAST = 3584
L = PAST + T            # 4096 == MAX_CACHE, nothing is trimmed
SCALE = 1.0 / np.sqrt(HD).astype(np.float32)
NCORES = 8
HPC = H // NCORES       # heads per core = 2
TOK = B * T             # 2048
NCH = L // 128          # 32 key chunks per (b, h)
PCH = PAST // 128       # 28 chunks from the cache
QC = T // 128           # 4 query chunks of 128
KO = D // 128           # 8 contraction steps for projections
FP32 = mybir.dt.float32
BF16 = mybir.dt.bfloat16
BF = ml_dtypes.bfloat16
NEG = -1.0e30

_cache = {}


def _build():
    nc = bacc.Bacc(None, target_bir_lowering=False)

    xT = nc.dram_tensor("xT", [D, TOK], BF16, kind="ExternalInput")
    # weights pre-swizzled on host to [p, ko, m] so DMA rows are 2KB
    wq = nc.dram_tensor("wq", [128, KO, 128], BF16, kind="ExternalInput")
    wk = nc.dram_tensor("wk", [128, KO, 128], BF16, kind="ExternalInput")
    wv = nc.dram_tensor("wv", [128, KO, 128], BF16, kind="ExternalInput")
    woT = nc.dram_tensor("woT", [128, D], BF16, kind="ExternalInput")
    kTp = nc.dram_tensor("kTp", [B, 128, PAST], BF16, kind="ExternalInput")
    vp = nc.dram_tensor("vp", [B, 128, HPC, PCH, HD + 1], BF16, kind="ExternalInput")
    out = nc.dram_tensor("out", [TOK, D], BF16, kind="ExternalOutput")

    Exp = mybir.ActivationFunctionType.Exp
    mult = mybir.AluOpType.mult

    # key-chunk groups: a scores psum tile holds up to 3 chunks (3 banks)
    groups = [list(range(s, min(s + 3, NCH))) for s in range(0, NCH, 3)]
    xT_r = xT.rearrange("(ko p) t -> p ko t", p=128)

    with tile.TileContext(nc) as tc:
        with (
            tc.tile_pool(name="const", bufs=1) as const,
            tc.tile_pool(name="persist", bufs=1) as persist,
            tc.tile_pool(name="kv", bufs=2) as kv,
            tc.tile_pool(name="pt", bufs=3) as ptp,
            tc.tile_pool(name="div", bufs=2) as divp,
            tc.tile_pool(name="osb", bufs=2) as osbp,
            tc.tile_pool(name="stage", bufs=4) as stage,
            tc.tile_pool(name="xw", bufs=1) as xw,
            tc.tile_pool(name="xs", bufs=2) as xs,
            tc.tile_pool(name="acc_ps", bufs=1, space="PSUM") as accp,
            tc.tile_pool(name="flex_ps", bufs=1, space="PSUM") as flexp,
            tc.tile_pool(name="sc_ps", bufs=2, space="PSUM") as scp,
        ):
            # ---- PE p-state warm-up: keep PE busy from t~0 so the clock
            # ramps to full speed before the first projection; operands are
            # uninitialized (outputs land in a psum bank nobody reads) ----
            dmy = const.tile([128, 128], BF16, tag="dmy")
            nc.gpsimd.memset(dmy, 0.0)
            dps = flexp.tile([128, 512], FP32, tag="flex", name="dps")
            for i in range(40):
                nc.tensor.matmul(
                    dps[:, :128], lhsT=dmy, rhs=dmy,
                    start=True, stop=True, skip_group_check=True,
                )

            # ---- DMAs next, ordered by first use ----
            w_s = {}
            for nm in ("q", "k", "v"):
                w_s[nm] = xw.tile([128, KO, 128], BF16, tag=f"w{nm}", name=f"w{nm}")
            xts, kts, vas = {}, {}, {}

            def load_x(bn, pieces=(4, 4)):
                xt = xs.tile([128, KO, 512], BF16, tag="xT", name=f"x{bn}")
                p0 = 0
                for np_ in pieces:
                    nc.sync.dma_start(
                        xt[:, p0 : p0 + np_, :],
                        xT_r[:, p0 : p0 + np_, bass.ts(bn, 512)],
                    )
                    p0 += np_
                xts[bn] = xt

            def load_kv(bn):
                kT = kv.tile([128, L], BF16, tag="kT", name=f"kT{bn}")
                nc.sync.dma_start(kT[:, : 6 * 128], kTp[bn, :, : 6 * 128])
                nc.sync.dma_start(kT[:, 6 * 128 : PAST], kTp[bn, :, 6 * 128 :])
                va = kv.tile([128, HPC, NCH, HD + 1], BF16, tag="va", name=f"va{bn}")
                nc.sync.dma_start(va[:, :, :PCH, :], vp[bn, :, :, :, :])
                nc.gpsimd.memset(va[:, :, PCH:, HD], 1.0)
                kts[bn], vas[bn] = kT, va

            nc.sync.dma_start(w_s["q"], wq[:, :, :])
            load_x(0, pieces=(4, 2, 1, 1))
            nc.sync.dma_start(w_s["k"], wk[:, :, :])
            load_kv(0)
            nc.sync.dma_start(w_s["v"], wv[:, :, :])
            load_x(1)
            woT_s = persist.tile([128, D], BF16)
            nc.sync.dma_start(woT_s, woT[:, :])

            # ---- constants ----
            identity = const.tile([128, 128], FP32)
            make_identity(nc, identity)
            ident_b = const.tile([128, 128], BF16)
            nc.vector.tensor_copy(ident_b, identity)
            masks = []
            for r in range(4):
                m = const.tile([128, T], FP32, tag=f"mask{r}", name=f"m{r}")
                nc.gpsimd.memset(m, 0.0)
                # keep 0 where query t >= key-token (128r + p), else NEG
                nc.gpsimd.affine_select(
                    out=m, in_=m, compare_op=mybir.AluOpType.is_ge,
                    fill=NEG, base=-128 * r, channel_multiplier=-1,
                    pattern=[[1, T]],
                )
                mb_ = const.tile([128, T], BF16, tag=f"maskb{r}", name=f"mb{r}")
                nc.vector.tensor_copy(mb_, m)
                masks.append(mb_)
            # (no warm-up exp needed: bacc inserts an explicit
            # LoadActFuncSet at the head of the program)

            # ---- persistent SBUF ----
            qT = persist.tile([128, TOK], BF16, tag="qT")
            oT = persist.tile([128, B, QC, 128], BF16, tag="oT")

            # ---- projection chores: <= ~430ns of PE work each ----
            def chores_qk(bn, which):
                """4 chores of 2 ko-steps; evict on the last."""
                box = {}

                def mk(piece):
                    def go():
                        if piece == 0:
                            box["ps"] = flexp.tile(
                                [128, 512], FP32, tag="flex", name=f"p{which}{bn}"
                            )
                        ps = box["ps"]
                        for ko in (2 * piece, 2 * piece + 1):
                            nc.tensor.matmul(
                                ps, lhsT=w_s[which][:, ko, :], rhs=xts[bn][:, ko, :],
                                start=(ko == 0), stop=(ko == KO - 1),
                                skip_group_check=True,
                            )
                        if piece == 3:
                            if which == "q":
                                nc.vector.tensor_copy(qT[:, bass.ts(bn, 512)], ps)
                            else:
                                nc.vector.tensor_copy(kts[bn][:, PAST:], ps)
                    return go

                return [mk(p) for p in range(4)]

            def chores_v(bn):
                """4 chores, one 128-token chunk each (8 matmuls of 128)."""
                def mk(tt):
                    def go():
                        psv = flexp.tile(
                            [128, HPC, HD], FP32, tag="flex", name=f"pv{bn}{tt}"
                        )
                        for ko in range(KO):
                            nc.tensor.matmul(
                                psv, lhsT=xts[bn][:, ko, bass.ts(tt, 128)],
                                rhs=w_s["v"][:, ko, :],
                                start=(ko == 0), stop=(ko == KO - 1),
                                skip_group_check=True,
                            )
                        nc.vector.tensor_copy(vas[bn][:, :, PCH + tt, :HD], psv)
                    return go

                return [mk(tt) for tt in range(QC)]

            osts = {}

            def chores_oproj(bn):
                """8 chores: (tt, nh) matmul + evict; DMA out after nh==1."""
                out_chores = []
                for tt in range(QC):
                    for nh in range(2):
                        def go(tt=tt, nh=nh):
                            ps = flexp.tile(
                                [128, 512], FP32, tag="flex", name=f"po{bn}{tt}{nh}"
                            )
                            nc.tensor.matmul(
                                ps, lhsT=oT[:, bn, tt, :],
                                rhs=woT_s[:, bass.ts(nh, 512)],
                                start=True, stop=True,
                            )
                            if nh == 0:
                                osts[(bn, tt)] = stage.tile(
                                    [128, D], BF16, tag="ost", name=f"ost{bn}{tt}"
                                )
                            ost = osts[(bn, tt)]
                            nc.vector.tensor_copy(ost[:, bass.ts(nh, 512)], ps)
                            if nh == 1:
                                r0 = bn * T + tt * 128
                                nc.sync.dma_start(out[r0 : r0 + 128, :], ost)
                        out_chores.append(go)
                return out_chores

            # one group of score matmuls (+ causal mask) and its exp
            def emit_group(bb, hh, gi):
                hsl = slice(hh * HD, (hh + 1) * HD)
                kT = kts[bb]
                g = groups[gi]
                sc = scp.tile([128, 3, 512], FP32, tag="sc", name=f"sc{bb}{hh}{gi}")
                for j, cc in enumerate(g):
                    r = cc - PCH
                    off = max(0, r * 128)
                    nc.tensor.matmul(
                        sc[:, j, off:],
                        lhsT=kT[hsl, bass.ts(cc, 128)],
                        rhs=qT[hsl, bb * T + off : (bb + 1) * T],
                        start=True, stop=r < 0,
                    )
                    if r >= 0:
                        # only the 128-wide triangle block needs masking
                        nc.tensor.matmul(
                            sc[:, j, off : off + 128],
                            lhsT=ident_b, rhs=masks[r][:, off : off + 128],
                            start=False, stop=True, skip_group_check=True,
                        )
                pT = ptp.tile([128, 3, 512], BF16, tag="pT", name=f"pT{bb}{hh}{gi}")
                if len(g) == 3:
                    nc.scalar.activation(pT, sc, Exp)
                else:  # last group: chunks 30 (q>=256) and 31 (q>=384)
                    nc.scalar.activation(pT[:, 0, 256:], sc[:, 0, 256:], Exp)
                    nc.scalar.activation(pT[:, 1, 384:], sc[:, 1, 384:], Exp)
                return pT

            # ---- attention pair: scores/exp run one group ahead of P.V;
            # the NEXT pair's first group is prefetched before this pair's
            # trailing work so the exp stream never gaps at pair boundaries
            def pair(b, h, osb, chores, ci, last=False, pre=None, nxt=None):
                va = vas[b]
                nxt_pre = None
                acc = accp.tile(
                    [128, QC, HD + 1], FP32, tag="acc", name=f"acc{b}{h}"
                )
                pts = {}

                def finish_qcs(b, h, osb, qlo, qhi):
                    """Last pair only: divide + transpose + o_proj + DMA for
                    query chunks [qlo, qhi) as soon as they stop accumulating.
                    The two chains run in parallel across PE/ScalarE/DVE
                    (the exp stream is done or nearly done here)."""
                    nqc = qhi - qlo
                    r_ = divp.tile([128, nqc], FP32, tag="r", name=f"rL{qlo}")
                    nc.vector.reciprocal(r_, acc[:, qlo:qhi, HD])
                    nc.vector.tensor_tensor(
                        osb[:, qlo:qhi, h, :], acc[:, qlo:qhi, :HD],
                        r_[:, :, None].to_broadcast([128, nqc, HD]), mult,
                    )
                    tps = {}
                    for qc in range(qlo, qhi):
                        # the final call's second transpose goes to the (then
                        # dead) acc bank so the chains don't serialize on the
                        # flex slot; earlier calls must NOT touch the acc bank
                        # (a start=True there wipes the open accumulations)
                        pool, tag = ((accp, "acc") if qc > qlo and qlo >= 2
                                     else (flexp, "flex"))
                        tp = pool.tile([128, 128], BF16, tag=tag,
                                       name=f"tpL{qc}")
                        nc.tensor.transpose(tp, osb[:, qc, :, :], ident_b)
                        tps[qc] = tp
                    for qc in range(qlo, qhi):
                        if qc == qlo:
                            nc.vector.tensor_copy(oT[:, b, qc, :], tps[qc])
                        else:
                            nc.scalar.copy(oT[:, b, qc, :], tps[qc])
                    # 4 o_proj matmuls into 4 DISTINCT psum banks (scA, scB,
                    # then the flex/acc banks that held the dead transposes)
                    # so no matmul WARs on an eviction
                    pss = {}
                    osts = {}
                    for qc in range(qlo, qhi):
                        osts[qc] = stage.tile([128, D], BF16, tag="ost",
                                              name=f"osL{qc}")
                    for nh in range(2):
                        for i, qc in enumerate(range(qlo, qhi)):
                            if nh == 0:
                                ps = scp.tile([128, 3, 512], FP32, tag="sc",
                                              name=f"pL{qc}{nh}")[:, 0, :]
                            else:
                                pool, tag = ((flexp, "flex") if i == 0
                                             else (accp, "acc"))
                                if qlo < 2 and i > 0:
                                    pool, tag = scp, "sc"
                                ps = pool.tile([128, 512], FP32, tag=tag,
                                               name=f"pL{qc}{nh}")
                                if tag == "sc":
                                    ps = ps[:, :512]
                            nc.tensor.matmul(
                                ps, lhsT=oT[:, b, qc, :],
                                rhs=woT_s[:, bass.ts(nh, 512)],
                                start=True, stop=True,
                            )
                            pss[(qc, nh)] = ps
                    # evicts: the early pair leans on ScalarE (its exp stream
                    # is ending), the last pair leans on DVE
                    for i, (qc, nh) in enumerate(
                        (q, n) for n in range(2) for q in range(qlo, qhi)
                    ):
                        ev = osts[qc][:, bass.ts(nh, 512)]
                        act_side = (i != 3) if qlo < 2 else (i >= 2)
                        if act_side:
                            nc.scalar.copy(ev, pss[(qc, nh)])
                        else:
                            nc.vector.tensor_copy(ev, pss[(qc, nh)])
                    for qc in range(qlo, qhi):
                        r0 = b * T + qc * 128
                        if qlo >= 2 and qc == qhi - 1:
                            # last row block: split per half so the first
                            # half's transfer overlaps the second's eviction
                            for nh in range(2):
                                nc.sync.dma_start(
                                    out[r0 : r0 + 128, bass.ts(nh, 512)],
                                    osts[qc][:, bass.ts(nh, 512)],
                                )
                        else:
                            nc.sync.dma_start(out[r0 : r0 + 128, :], osts[qc])

                def pv(gi):
                    pT = pts[gi]
                    for j, cc in enumerate(groups[gi]):
                        for qc in range(max(0, cc - PCH), QC):
                            # one start/stop per psum bank per pair: start
                            # marks the WHOLE 2KB bank pending-zero, so each
                            # region's first write overwrites (self-zeroing)
                            nc.tensor.matmul(
                                acc[:, qc, :],
                                lhsT=pT[:, j, bass.ts(qc, 128)],
                                rhs=va[:, h, cc, :],
                                start=(cc == 0 and qc == 0),
                                stop=(cc == NCH - 1 and qc == QC - 1),
                                skip_group_check=True,
                            )

                npre = len(pre) if pre is not None else 0
                for gi in range(len(groups)):
                    if gi < npre:
                        pts[gi] = pre[gi]
                    else:
                        pts[gi] = emit_group(b, h, gi)
                    if gi == len(groups) - 1 and nxt is not None:
                        nxt_pre = (emit_group(nxt[0], nxt[1], 0),)
                    if gi > 0:
                        pv(gi - 1)
                    if ci < len(chores) and gi >= npre:
                        chores[ci]()
                        ci += 1
                    if last and gi == len(groups) - 1:
                        # qc 0/1 stopped at chunks 28/29 (group 9): finish
                        # them while ScalarE still runs the last exps
                        finish_qcs(b, h, osb, 0, 2)
                pv(len(groups) - 1)

                if last:
                    finish_qcs(b, h, osb, 2, 4)
                else:
                    # divide by the softmax denominator (column 64 of acc)
                    r_ = divp.tile([128, QC], FP32, tag="r", name=f"r{b}{h}")
                    nc.vector.reciprocal(r_, acc[:, :, HD])
                    nc.vector.tensor_tensor(
                        osb[:, :, h, :], acc[:, :, :HD],
                        r_[:, :, None].to_broadcast([128, QC, HD]), mult,
                    )
                return ci, nxt_pre

            def finish_batch(b, osb):
                # o [q, ch] -> oT [ch, q] per query chunk (both heads at once)
                tp = accp.tile([128, QC, 128], BF16, tag="acc", name=f"tp{b}")
                for qc in range(QC):
                    nc.tensor.transpose(tp[:, qc, :], osb[:, qc, :, :], ident_b)
                nc.vector.tensor_copy(oT[:, b], tp)

            # ---- prologue: only batch-0 q projection (ScalarE idle) ----
            ps_q = flexp.tile([128, 512], FP32, tag="flex", name="psq0")
            for ko in range(KO):
                nc.tensor.matmul(
                    ps_q, lhsT=w_s["q"][:, ko, :], rhs=xts[0][:, ko, :],
                    start=(ko == 0), stop=(ko == KO - 1), skip_group_check=True,
                )
            nc.vector.tensor_copy(qT[:, :512], ps_q)

            # ---- main loop ----
            pre = None
            for b in range(B):
                if b + 1 < B:
                    load_kv(b + 1)
                chores = []
                if b == 0:
                    chores += chores_qk(0, "k") + chores_v(0)
                else:
                    chores += chores_oproj(b - 1)
                if b + 1 < B:
                    chores += chores_qk(b + 1, "q") + chores_qk(b + 1, "k")
                    chores += chores_v(b + 1)
                osb = osbp.tile([128, QC, HPC, HD], BF16, tag="osb", name=f"osb{b}")
                ci, pre = pair(b, 0, osb, chores, 0, pre=pre, nxt=(b, 1))
                if b + 2 < B:
                    load_x(b + 2)
                nxt = (b + 1, 0) if b + 1 < B else None
                ci, pre = pair(
                    b, 1, osb, chores, ci, last=(b == B - 1), pre=pre, nxt=nxt
                )
                assert ci >= len(chores), (b, ci, len(chores))
                if b < B - 1:
                    finish_batch(b, osb)

    nc.compile()
    return nc


def _pack_v(v):
    """[B, HPC, PAST, HD] -> [B, 128, HPC, PCH, HD+1] bf16, ones in col HD."""
    o = np.empty((B, 128, HPC, PCH, HD + 1), np.float32)
    o[..., :HD] = v.reshape(B, HPC, PCH, 128, HD).transpose(0, 3, 1, 2, 4)
    o[..., HD] = 1.0
    return np.ascontiguousarray(o.astype(BF))


def _prep(x, k_prev, v_prev, Wq, Wk, Wv, Wo):
    """Host-side shard + layout marshalling (bf16, C-contiguous)."""
    f = np.float32
    x2 = np.asarray(x, f).reshape(TOK, D)
    xT = np.ascontiguousarray(x2.T.astype(BF))
    k_prev = np.asarray(k_prev, f)
    v_prev = np.asarray(v_prev, f)
    Wq, Wk, Wv, Wo = (np.asarray(w, f) for w in (Wq, Wk, Wv, Wo))

    def swz(wT):
        # [D, 128] -> [p, ko, m] with row (ko*128+p) -> [p, ko, :]
        return np.ascontiguousarray(
            wT.reshape(KO, 128, 128).transpose(1, 0, 2).astype(BF)
        )

    in_maps = []
    for c in range(NCORES):
        rows = slice(128 * c, 128 * (c + 1))
        hsl = slice(HPC * c, HPC * (c + 1))
        in_maps.append(
            {
                "xT": xT,
                "wq": swz((Wq[rows, :] * SCALE).T),
                "wk": swz(Wk[rows, :].T),
                "wv": swz(Wv[rows, :].T),
                "woT": np.ascontiguousarray(Wo[:, rows].T.astype(BF)),
                "kTp": np.ascontiguousarray(
                    k_prev[:, hsl, :, :].transpose(0, 1, 3, 2).astype(BF)
                ).reshape(B, 128, PAST),
                "vp": _pack_v(v_prev[:, hsl, :, :]),
            }
        )
    return in_maps


def kernel(x, k_prev, v_prev, Wq, Wk, Wv, Wo):
    if "nc" not in _cache:
        _cache["nc"] = _build()
    nc = _cache["nc"]
    in_maps = _prep(x, k_prev, v_prev, Wq, Wk, Wv, Wo)
    res = run_bass_kernel_spmd(nc, in_maps, core_ids=list(range(NCORES)))
    acc = np.zeros((TOK, D), np.float64)
    for r in res.results:
        acc += np.asarray(r["out"], np.float32)
    return acc.astype(np.float32).reshape(B, T, D)


# revision 43
# speedup vs baseline: 1.3148x; 1.0052x over previous
"""Multi-head attention with KV cache, sharded over 8 NeuronCores by head.

Problem (hardcoded shapes):
  x       [4, 512, 1024]      hidden states (B, T, D)
  k_prev  [4, 16, 3584, 64]   KV cache (B, H, PAST, HD)
  v_prev  [4, 16, 3584, 64]
  Wq/Wk/Wv/Wo [1024, 1024]    projection weights (torch Linear: y = x @ W.T)

Sharding: 16 heads / 8 cores = 2 heads per core; each core computes its
2 heads' q/k/v projections, attention, and a column-parallel o_proj
partial [2048, 1024]; the host sums the 8 partials.

Device algorithm (all matmul operands bf16, fp32 PSUM accumulate;
measured end-to-end rel err ~1e-3 vs the 2e-2 gate):
  - q^T/k^T = W_slice @ x^T (contract D over 8 ko-steps); v computed
    directly in [token, channel] orientation so it lands in the
    [key-partition, head-dim] layout attention needs (no PE transposes).
  - scores^T[key, q] per 128-key chunk (K=HD=64 on partitions); causal
    mask on the 4 newest chunks added via identity @ mask matmul; exp on
    ScalarE (groups of 3 chunks = 3 PSUM banks, double buffered).
  - P.V in [query-partition, head-dim] orientation: out[q, hd] += over
    key chunks with lhsT = pT chunk [keys, q], rhs = [v | 1] [keys, 65]
    so column 64 accumulates the softmax denominator. 65-wide outputs
    cost 65 PE cycles/matmul vs 512 for the transposed orientation.
  - divide by denominator on DVE (free-dim broadcast), PE-transpose o to
    [channel, token] for o_proj, o_proj column-parallel, bf16 out.
  Scheduling: exp is the critical path (~123us on ScalarE). The P.V
  matmuls trail the score matmuls by one group so PE never blocks the
  exp stream, and projection/o_proj work is split into <=450ns "chores"
  threaded one-per-group between attention groups.
"""

import numpy as np
import ml_dtypes

import concourse.bass as bass
import concourse.mybir as mybir
import concourse.tile as tile
from concourse import bacc
from concourse.bass_utils import run_bass_kernel_spmd
from concourse.masks import make_identity

B, T, D = 4, 512, 1024
H, HD = 16, 64
PAST = 3584
L = PAST + T            # 4096 == MAX_CACHE, nothing is trimmed
SCALE = 1.0 / np.sqrt(HD).astype(np.float32)
NCORES = 8
HPC = H // NCORES       # heads per core = 2
TOK = B * T             # 2048
NCH = L // 128          # 32 key chunks per (b, h)
PCH = PAST // 128       # 28 chunks from the cache
QC = T // 128           # 4 query chunks of 128
KO = D // 128           # 8 contraction steps for projections
FP32 = mybir.dt.float32
BF16 = mybir.dt.bfloat16
BF = ml_dtypes.bfloat16
NEG = -1.0e30

_cache = {}


def _build():
    nc = bacc.Bacc(None, target_bir_lowering=False)

    xT = nc.dram_tensor("xT", [D, TOK], BF16, kind="ExternalInput")
    # weights pre-swizzled on host to [p, ko, m] so DMA rows are 2KB
    wq = nc.dram_tensor("wq", [128, KO, 128], BF16, kind="ExternalInput")
    wk = nc.dram_tensor("wk", [128, KO, 128], BF16, kind="ExternalInput")
    wv = nc.dram_tensor("wv", [128, KO, 128], BF16, kind="ExternalInput")
    woT = nc.dram_tensor("woT", [128, D], BF16, kind="ExternalInput")
    kTp = nc.dram_tensor("kTp", [B, 128, PAST], BF16, kind="ExternalInput")
    vp = nc.dram_tensor("vp", [B, 128, HPC, PCH, HD + 1], BF16, kind="ExternalInput")
    out = nc.dram_tensor("out", [TOK, D], BF16, kind="ExternalOutput")

    Exp = mybir.ActivationFunctionType.Exp
    mult = mybir.AluOpType.mult

    # key-chunk groups: a scores psum tile holds up to 3 chunks (3 banks)
    groups = [list(range(s, min(s + 3, NCH))) for s in range(0, NCH, 3)]
    xT_r = xT.rearrange("(ko p) t -> p ko t", p=128)

    with tile.TileContext(nc) as tc:
        with (
            tc.tile_pool(name="const", bufs=1) as const,
            tc.tile_pool(name="persist", bufs=1) as persist,
            tc.tile_pool(name="kv", bufs=2) as kv,
            tc.tile_pool(name="pt", bufs=3) as ptp,
            tc.tile_pool(name="div", bufs=2) as divp,
            tc.tile_pool(name="osb", bufs=2) as osbp,
            tc.tile_pool(name="stage", bufs=4) as stage,
            tc.tile_pool(name="xw", bufs=1) as xw,
            tc.tile_pool(name="xs", bufs=2) as xs,
            tc.tile_pool(name="acc_ps", bufs=1, space="PSUM") as accp,
            tc.tile_pool(name="flex_ps", bufs=1, space="PSUM") as flexp,
            tc.tile_pool(name="sc_ps", bufs=2, space="PSUM") as scp,
        ):
            # ---- PE p-state warm-up: keep PE busy from t~0 so the clock
            # ramps to full speed before the first projection; operands are
            # uninitialized (outputs land in a psum bank nobody reads) ----
            dmy = const.tile([128, 128], BF16, tag="dmy")
            nc.gpsimd.memset(dmy, 0.0)
            dps = flexp.tile([128, 512], FP32, tag="flex", name="dps")
            for i in range(40):
                nc.tensor.matmul(
                    dps[:, :128], lhsT=dmy, rhs=dmy,
                    start=True, stop=True, skip_group_check=True,
                )

            # ---- DMAs next, ordered by first use ----
            w_s = {}
            for nm in ("q", "k", "v"):
                w_s[nm] = xw.tile([128, KO, 128], BF16, tag=f"w{nm}", name=f"w{nm}")
            xts, kts, vas = {}, {}, {}

            def load_x(bn, pieces=(4, 4)):
                xt = xs.tile([128, KO, 512], BF16, tag="xT", name=f"x{bn}")
                p0 = 0
                for np_ in pieces:
                    nc.sync.dma_start(
                        xt[:, p0 : p0 + np_, :],
                        xT_r[:, p0 : p0 + np_, bass.ts(bn, 512)],
                    )
                    p0 += np_
                xts[bn] = xt

            def load_kv(bn):
                kT = kv.tile([128, L], BF16, tag="kT", name=f"kT{bn}")
                nc.sync.dma_start(kT[:, : 6 * 128], kTp[bn, :, : 6 * 128])
                nc.sync.dma_start(kT[:, 6 * 128 : PAST], kTp[bn, :, 6 * 128 :])
                va = kv.tile([128, HPC, NCH, HD + 1], BF16, tag="va", name=f"va{bn}")
                nc.sync.dma_start(va[:, :, :PCH, :], vp[bn, :, :, :, :])
                nc.gpsimd.memset(va[:, :, PCH:, HD], 1.0)
                kts[bn], vas[bn] = kT, va

            nc.sync.dma_start(w_s["q"], wq[:, :, :])
            load_x(0, pieces=(4, 2, 1, 1))
            nc.sync.dma_start(w_s["k"], wk[:, :, :])
            load_kv(0)
            nc.sync.dma_start(w_s["v"], wv[:, :, :])
            load_x(1)
            woT_s = persist.tile([128, D], BF16)
            nc.sync.dma_start(woT_s, woT[:, :])

            # ---- constants ----
            identity = const.tile([128, 128], FP32)
            make_identity(nc, identity)
            ident_b = const.tile([128, 128], BF16)
            nc.vector.tensor_copy(ident_b, identity)
            masks = []
            for r in range(4):
                m = const.tile([128, T], FP32, tag=f"mask{r}", name=f"m{r}")
                nc.gpsimd.memset(m, 0.0)
                # keep 0 where query t >= key-token (128r + p), else NEG
                nc.gpsimd.affine_select(
                    out=m, in_=m, compare_op=mybir.AluOpType.is_ge,
                    fill=NEG, base=-128 * r, channel_multiplier=-1,
                    pattern=[[1, T]],
                )
                mb_ = const.tile([128, T], BF16, tag=f"maskb{r}", name=f"mb{r}")
                nc.vector.tensor_copy(mb_, m)
                masks.append(mb_)
            # (no warm-up exp needed: bacc inserts an explicit
            # LoadActFuncSet at the head of the program)

            # ---- persistent SBUF ----
            qT = persist.tile([128, TOK], BF16, tag="qT")
            oT = persist.tile([128, B, QC, 128], BF16, tag="oT")

            # ---- projection chores: <= ~430ns of PE work each ----
            def chores_qk(bn, which):
                """4 chores of 2 ko-steps; evict on the last."""
                box = {}

                def mk(piece):
                    def go():
                        if piece == 0:
                            box["ps"] = flexp.tile(
                                [128, 512], FP32, tag="flex", name=f"p{which}{bn}"
                            )
                        ps = box["ps"]
                        for ko in (2 * piece, 2 * piece + 1):
                            nc.tensor.matmul(
                                ps, lhsT=w_s[which][:, ko, :], rhs=xts[bn][:, ko, :],
                                start=(ko == 0), stop=(ko == KO - 1),
                                skip_group_check=True,
                            )
                        if piece == 3:
                            if which == "q":
                                nc.vector.tensor_copy(qT[:, bass.ts(bn, 512)], ps)
                            else:
                                nc.vector.tensor_copy(kts[bn][:, PAST:], ps)
                    return go

                return [mk(p) for p in range(4)]

            def chores_v(bn):
                """4 chores, one 128-token chunk each (8 matmuls of 128)."""
                def mk(tt):
                    def go():
                        psv = flexp.tile(
                            [128, HPC, HD], FP32, tag="flex", name=f"pv{bn}{tt}"
                        )
                        for ko in range(KO):
                            nc.tensor.matmul(
                                psv, lhsT=xts[bn][:, ko, bass.ts(tt, 128)],
                                rhs=w_s["v"][:, ko, :],
                                start=(ko == 0), stop=(ko == KO - 1),
                                skip_group_check=True,
                            )
                        nc.vector.tensor_copy(vas[bn][:, :, PCH + tt, :HD], psv)
                    return go

                return [mk(tt) for tt in range(QC)]

            osts = {}

            def chores_oproj(bn):
                """8 chores: (tt, nh) matmul + evict; DMA out after nh==1."""
                out_chores = []
                for tt in range(QC):
                    for nh in range(2):
                        def go(tt=tt, nh=nh):
                            ps = flexp.tile(
                                [128, 512], FP32, tag="flex", name=f"po{bn}{tt}{nh}"
                            )
                            nc.tensor.matmul(
                                ps, lhsT=oT[:, bn, tt, :],
                                rhs=woT_s[:, bass.ts(nh, 512)],
                                start=True, stop=True,
                            )
                            if nh == 0:
                                osts[(bn, tt)] = stage.tile(
                                    [128, D], BF16, tag="ost", name=f"ost{bn}{tt}"
                                )
                            ost = osts[(bn, tt)]
                            nc.vector.tensor_copy(ost[:, bass.ts(nh, 512)], ps)
                            if nh == 1:
                                r0 = bn * T + tt * 128
                                nc.sync.dma_start(out[r0 : r0 + 128, :], ost)
                        out_chores.append(go)
                return out_chores

            # one group of score matmuls (+ causal mask) and ONE exp.
            # Masked chunks are packed densely (no garbage columns in the
            # exp): base[cc] is the packed column offset of chunk cc's
            # visible queries [off, 512). The last (2-chunk, 384-column)
            # group lives in the flex bank so the score ring frees early.
            def emit_group(bb, hh, gi):
                hsl = slice(hh * HD, (hh + 1) * HD)
                kT = kts[bb]
                g = groups[gi]
                sc = scp.tile([128, 1536], FP32, tag="sc",
                              name=f"sc{bb}{hh}{gi}")
                if gi == len(groups) - 1:
                    W = 384
                elif gi == len(groups) - 2:
                    W = 1408
                else:
                    W = 512 * len(g)
                bases = {}
                base = 0
                for cc in g:
                    r = cc - PCH
                    off = max(0, r * 128)
                    bases[cc] = (base, off)
                    nc.tensor.matmul(
                        sc[:, base : base + 512 - off],
                        lhsT=kT[hsl, bass.ts(cc, 128)],
                        rhs=qT[hsl, bb * T + off : (bb + 1) * T],
                        start=True, stop=r < 0,
                    )
                    if r >= 0:
                        # only the 128-wide triangle block needs masking
                        nc.tensor.matmul(
                            sc[:, base : base + 128],
                            lhsT=ident_b, rhs=masks[r][:, off : off + 128],
                            start=False, stop=True, skip_group_check=True,
                        )
                    base += 512 - off
                assert base == W, (gi, base, W)
                pT = ptp.tile([128, 1536], BF16, tag="pT", name=f"pT{bb}{hh}{gi}")
                nc.scalar.activation(pT[:, :W], sc[:, :W], Exp)
                return pT, bases

            # ---- attention pair: scores/exp run one group ahead of P.V;
            # the NEXT pair's first group is prefetched before this pair's
            # trailing work so the exp stream never gaps at pair boundaries
            def pair(b, h, osb, chores, ci, last=False, pre=None, nxt=None):
                va = vas[b]
                nxt_pre = None
                acc = accp.tile(
                    [128, QC, HD + 1], FP32, tag="acc", name=f"acc{b}{h}"
                )
                pts = {}

                def finish_qcs(b, h, osb, qlo, qhi):
                    """Last pair only: divide + transpose + o_proj + DMA for
                    query chunks [qlo, qhi) as soon as they stop accumulating.
                    The two chains run in parallel across PE/ScalarE/DVE
                    (the exp stream is done or nearly done here)."""
                    nqc = qhi - qlo
                    r_ = divp.tile([128, nqc], FP32, tag="r", name=f"rL{qlo}")
                    nc.vector.reciprocal(r_, acc[:, qlo:qhi, HD])
                    nc.vector.tensor_tensor(
                        osb[:, qlo:qhi, h, :], acc[:, qlo:qhi, :HD],
                        r_[:, :, None].to_broadcast([128, nqc, HD]), mult,
                    )
                    tps = {}
                    for qc in range(qlo, qhi):
                        # the final call's second transpose goes to the (then
                        # dead) acc bank so the chains don't serialize on the
                        # flex slot; earlier calls must NOT touch the acc bank
                        # (a start=True there wipes the open accumulations)
                        pool, tag = ((accp, "acc") if qc > qlo and qlo >= 2
                                     else (flexp, "flex"))
                        tp = pool.tile([128, 128], BF16, tag=tag,
                                       name=f"tpL{qc}")
                        nc.tensor.transpose(tp, osb[:, qc, :, :], ident_b)
                        tps[qc] = tp
                    for qc in range(qlo, qhi):
                        if qc == qlo:
                            nc.vector.tensor_copy(oT[:, b, qc, :], tps[qc])
                        else:
                            nc.scalar.copy(oT[:, b, qc, :], tps[qc])
                    # 4 o_proj matmuls into 4 DISTINCT psum banks (scA, scB,
                    # then the flex/acc banks that held the dead transposes)
                    # so no matmul WARs on an eviction
                    pss = {}
                    osts = {}
                    for qc in range(qlo, qhi):
                        osts[qc] = stage.tile([128, D], BF16, tag="ost",
                                              name=f"osL{qc}")
                    for nh in range(2):
                        for i, qc in enumerate(range(qlo, qhi)):
                            if nh == 0:
                                ps = scp.tile([128, 3, 512], FP32, tag="sc",
                                              name=f"pL{qc}{nh}")[:, 0, :]
                            else:
                                pool, tag = ((flexp, "flex") if i == 0
                                             else (accp, "acc"))
                                if qlo < 2 and i > 0:
                                    pool, tag = scp, "sc"
                                ps = pool.tile([128, 512], FP32, tag=tag,
                                               name=f"pL{qc}{nh}")
                                if tag == "sc":
                                    ps = ps[:, :512]
                            nc.tensor.matmul(
                                ps, lhsT=oT[:, b, qc, :],
                                rhs=woT_s[:, bass.ts(nh, 512)],
                                start=True, stop=True,
                            )
                            pss[(qc, nh)] = ps
                    # evicts: the early pair leans on ScalarE (its exp stream
                    # is ending), the last pair leans on DVE
                    for i, (qc, nh) in enumerate(
                        (q, n) for n in range(2) for q in range(qlo, qhi)
                    ):
                        ev = osts[qc][:, bass.ts(nh, 512)]
                        act_side = (i != 3) if qlo < 2 else (i >= 2)
                        if act_side:
                            nc.scalar.copy(ev, pss[(qc, nh)])
                        else:
                            nc.vector.tensor_copy(ev, pss[(qc, nh)])
                    for qc in range(qlo, qhi):
                        r0 = b * T + qc * 128
                        if qlo >= 2 and qc == qhi - 1:
                            # last row block: split per half so the first
                            # half's transfer overlaps the second's eviction
                            for nh in range(2):
                                nc.sync.dma_start(
                                    out[r0 : r0 + 128, bass.ts(nh, 512)],
                                    osts[qc][:, bass.ts(nh, 512)],
                                )
                        else:
                            nc.sync.dma_start(out[r0 : r0 + 128, :], osts[qc])

                def pv(gi):
                    pT, bases = pts[gi]
                    for cc in groups[gi]:
                        base, off = bases[cc]
                        for qc in range(max(0, cc - PCH), QC):
                            c0 = base + qc * 128 - off
                            # one start/stop per psum bank per pair: start
                            # marks the WHOLE 2KB bank pending-zero, so each
                            # region's first write overwrites (self-zeroing)
                            nc.tensor.matmul(
                                acc[:, qc, :],
                                lhsT=pT[:, c0 : c0 + 128],
                                rhs=va[:, h, cc, :],
                                start=(cc == 0 and qc == 0),
                                stop=(cc == NCH - 1 and qc == QC - 1),
                                skip_group_check=True,
                            )

                npre = len(pre) if pre is not None else 0
                for gi in range(len(groups)):
                    if gi < npre:
                        pts[gi] = pre[gi]
                    else:
                        pts[gi] = emit_group(b, h, gi)
                    if gi == len(groups) - 1 and nxt is not None:
                        nxt_pre = (emit_group(nxt[0], nxt[1], 0),)
                    if gi > 0:
                        pv(gi - 1)
                    if ci < len(chores) and gi >= npre:
                        chores[ci]()
                        ci += 1
                    if last and gi == len(groups) - 1:
                        # qc 0/1 stopped at chunks 28/29 (group 9): finish
                        # them while ScalarE still runs the last exps
                        finish_qcs(b, h, osb, 0, 2)
                pv(len(groups) - 1)
                if ci < len(chores):
                    chores[ci]()
                    ci += 1

                if last:
                    finish_qcs(b, h, osb, 2, 4)
                else:
                    # divide by the softmax denominator (column 64 of acc)
                    r_ = divp.tile([128, QC], FP32, tag="r", name=f"r{b}{h}")
                    nc.vector.reciprocal(r_, acc[:, :, HD])
                    nc.vector.tensor_tensor(
                        osb[:, :, h, :], acc[:, :, :HD],
                        r_[:, :, None].to_broadcast([128, QC, HD]), mult,
                    )
                return ci, nxt_pre

            def finish_batch(b, osb):
                # o [q, ch] -> oT [ch, q] per query chunk (both heads at once)
                tp = accp.tile([128, QC, 128], BF16, tag="acc", name=f"tp{b}")
                for qc in range(QC):
                    nc.tensor.transpose(tp[:, qc, :], osb[:, qc, :, :], ident_b)
                nc.vector.tensor_copy(oT[:, b], tp)

            # ---- prologue: only batch-0 q projection (ScalarE idle) ----
            ps_q = flexp.tile([128, 512], FP32, tag="flex", name="psq0")
            for ko in range(KO):
                nc.tensor.matmul(
                    ps_q, lhsT=w_s["q"][:, ko, :], rhs=xts[0][:, ko, :],
                    start=(ko == 0), stop=(ko == KO - 1), skip_group_check=True,
                )
            nc.vector.tensor_copy(qT[:, :512], ps_q)

            # ---- main loop ----
            pre = None
            for b in range(B):
                if b + 1 < B:
                    load_kv(b + 1)
                chores = []
                if b == 0:
                    chores += chores_qk(0, "k") + chores_v(0)
                else:
                    chores += chores_oproj(b - 1)
                if b + 1 < B:
                    chores += chores_qk(b + 1, "q") + chores_qk(b + 1, "k")
                    chores += chores_v(b + 1)
                osb = osbp.tile([128, QC, HPC, HD], BF16, tag="osb", name=f"osb{b}")
                ci, pre = pair(b, 0, osb, chores, 0, pre=pre, nxt=(b, 1))
                if b + 2 < B:
                    load_x(b + 2)
                nxt = (b + 1, 0) if b + 1 < B else None
                ci, pre = pair(
                    b, 1, osb, chores, ci, last=(b == B - 1), pre=pre, nxt=nxt
                )
                assert ci >= len(chores), (b, ci, len(chores))
                if b < B - 1:
                    finish_batch(b, osb)

    nc.compile()
    return nc


def _pack_v(v):
    """[B, HPC, PAST, HD] -> [B, 128, HPC, PCH, HD+1] bf16, ones in col HD."""
    o = np.empty((B, 128, HPC, PCH, HD + 1), np.float32)
    o[..., :HD] = v.reshape(B, HPC, PCH, 128, HD).transpose(0, 3, 1, 2, 4)
    o[..., HD] = 1.0
    return np.ascontiguousarray(o.astype(BF))


def _prep(x, k_prev, v_prev, Wq, Wk, Wv, Wo):
    """Host-side shard + layout marshalling (bf16, C-contiguous)."""
    f = np.float32
    x2 = np.asarray(x, f).reshape(TOK, D)
    xT = np.ascontiguousarray(x2.T.astype(BF))
    k_prev = np.asarray(k_prev, f)
    v_prev = np.asarray(v_prev, f)
    Wq, Wk, Wv, Wo = (np.asarray(w, f) for w in (Wq, Wk, Wv, Wo))

    def swz(wT):
        # [D, 128] -> [p, ko, m] with row (ko*128+p) -> [p, ko, :]
        return np.ascontiguousarray(
            wT.reshape(KO, 128, 128).transpose(1, 0, 2).astype(BF)
        )

    in_maps = []
    for c in range(NCORES):
        rows = slice(128 * c, 128 * (c + 1))
        hsl = slice(HPC * c, HPC * (c + 1))
        in_maps.append(
            {
                "xT": xT,
                "wq": swz((Wq[rows, :] * SCALE).T),
                "wk": swz(Wk[rows, :].T),
                "wv": swz(Wv[rows, :].T),
                "woT": np.ascontiguousarray(Wo[:, rows].T.astype(BF)),
                "kTp": np.ascontiguousarray(
                    k_prev[:, hsl, :, :].transpose(0, 1, 3, 2).astype(BF)
                ).reshape(B, 128, PAST),
                "vp": _pack_v(v_prev[:, hsl, :, :]),
            }
        )
    return in_maps


def kernel(x, k_prev, v_prev, Wq, Wk, Wv, Wo):
    if "nc" not in _cache:
        _cache["nc"] = _build()
    nc = _cache["nc"]
    in_maps = _prep(x, k_prev, v_prev, Wq, Wk, Wv, Wo)
    res = run_bass_kernel_spmd(nc, in_maps, core_ids=list(range(NCORES)))
    acc = np.zeros((TOK, D), np.float64)
    for r in res.results:
        acc += np.asarray(r["out"], np.float32)
    return acc.astype(np.float32).reshape(B, T, D)


# revision 46
# speedup vs baseline: 1.3155x; 1.0005x over previous
"""Multi-head attention with KV cache, sharded over 8 NeuronCores by head.

Problem (hardcoded shapes):
  x       [4, 512, 1024]      hidden states (B, T, D)
  k_prev  [4, 16, 3584, 64]   KV cache (B, H, PAST, HD)
  v_prev  [4, 16, 3584, 64]
  Wq/Wk/Wv/Wo [1024, 1024]    projection weights (torch Linear: y = x @ W.T)

Sharding: 16 heads / 8 cores = 2 heads per core; each core computes its
2 heads' q/k/v projections, attention, and a column-parallel o_proj
partial [2048, 1024]; the host sums the 8 partials.

Device algorithm (all matmul operands bf16, fp32 PSUM accumulate;
measured end-to-end rel err ~1e-3 vs the 2e-2 gate):
  - q^T/k^T = W_slice @ x^T (contract D over 8 ko-steps); v computed
    directly in [token, channel] orientation so it lands in the
    [key-partition, head-dim] layout attention needs (no PE transposes).
  - scores^T[key, q] per 128-key chunk (K=HD=64 on partitions); causal
    mask on the 4 newest chunks added via identity @ mask matmul; exp on
    ScalarE (groups of 3 chunks = 3 PSUM banks, double buffered).
  - P.V in [query-partition, head-dim] orientation: out[q, hd] += over
    key chunks with lhsT = pT chunk [keys, q], rhs = [v | 1] [keys, 65]
    so column 64 accumulates the softmax denominator. 65-wide outputs
    cost 65 PE cycles/matmul vs 512 for the transposed orientation.
  - divide by denominator on DVE (free-dim broadcast), PE-transpose o to
    [channel, token] for o_proj, o_proj column-parallel, bf16 out.
  Scheduling: exp is the critical path (~123us on ScalarE). The P.V
  matmuls trail the score matmuls by one group so PE never blocks the
  exp stream, and projection/o_proj work is split into <=450ns "chores"
  threaded one-per-group between attention groups.
"""

import numpy as np
import ml_dtypes

import concourse.bass as bass
import concourse.mybir as mybir
import concourse.tile as tile
from concourse import bacc
from concourse.bass_utils import run_bass_kernel_spmd
from concourse.masks import make_identity

B, T, D = 4, 512, 1024
H, HD = 16, 64
PAST = 3584
L = PAST + T            # 4096 == MAX_CACHE, nothing is trimmed
SCALE = 1.0 / np.sqrt(HD).astype(np.float32)
NCORES = 8
HPC = H // NCORES       # heads per core = 2
TOK = B * T             # 2048
NCH = L // 128          # 32 key chunks per (b, h)
PCH = PAST // 128       # 28 chunks from the cache
QC = T // 128           # 4 query chunks of 128
KO = D // 128           # 8 contraction steps for projections
FP32 = mybir.dt.float32
BF16 = mybir.dt.bfloat16
BF = ml_dtypes.bfloat16
NEG = -1.0e30

_cache = {}


def _build():
    nc = bacc.Bacc(None, target_bir_lowering=False)

    xT = nc.dram_tensor("xT", [D, TOK], BF16, kind="ExternalInput")
    # weights pre-swizzled on host to [p, ko, m] so DMA rows are 2KB
    wq = nc.dram_tensor("wq", [128, KO, 128], BF16, kind="ExternalInput")
    wk = nc.dram_tensor("wk", [128, KO, 128], BF16, kind="ExternalInput")
    wv = nc.dram_tensor("wv", [128, KO, 128], BF16, kind="ExternalInput")
    woT = nc.dram_tensor("woT", [128, D], BF16, kind="ExternalInput")
    kTp = nc.dram_tensor("kTp", [B, 128, PAST], BF16, kind="ExternalInput")
    vp = nc.dram_tensor("vp", [B, 128, HPC, PCH, HD + 1], BF16, kind="ExternalInput")
    out = nc.dram_tensor("out", [TOK, D], BF16, kind="ExternalOutput")

    Exp = mybir.ActivationFunctionType.Exp
    mult = mybir.AluOpType.mult

    # key-chunk groups: a scores psum tile holds up to 3 chunks (3 banks)
    groups = [list(range(s, min(s + 3, NCH))) for s in range(0, NCH, 3)]
    xT_r = xT.rearrange("(ko p) t -> p ko t", p=128)

    with tile.TileContext(nc) as tc:
        with (
            tc.tile_pool(name="const", bufs=1) as const,
            tc.tile_pool(name="persist", bufs=1) as persist,
            tc.tile_pool(name="kv", bufs=2) as kv,
            tc.tile_pool(name="pt", bufs=4) as ptp,
            tc.tile_pool(name="div", bufs=2) as divp,
            tc.tile_pool(name="osb", bufs=2) as osbp,
            tc.tile_pool(name="stage", bufs=4) as stage,
            tc.tile_pool(name="xw", bufs=1) as xw,
            tc.tile_pool(name="xs", bufs=2) as xs,
            tc.tile_pool(name="acc_ps", bufs=1, space="PSUM") as accp,
            tc.tile_pool(name="flex_ps", bufs=1, space="PSUM") as flexp,
            tc.tile_pool(name="sc_ps", bufs=2, space="PSUM") as scp,
        ):
            # ---- PE p-state warm-up: keep PE busy from t~0 so the clock
            # ramps to full speed before the first projection; operands are
            # uninitialized (outputs land in a psum bank nobody reads) ----
            dmy = const.tile([128, 128], BF16, tag="dmy")
            nc.gpsimd.memset(dmy, 0.0)
            dps = flexp.tile([128, 512], FP32, tag="flex", name="dps")
            for i in range(40):
                nc.tensor.matmul(
                    dps[:, :128], lhsT=dmy, rhs=dmy,
                    start=True, stop=True, skip_group_check=True,
                )

            # ---- DMAs next, ordered by first use ----
            w_s = {}
            for nm in ("q", "k", "v"):
                w_s[nm] = xw.tile([128, KO, 128], BF16, tag=f"w{nm}", name=f"w{nm}")
            xts, kts, vas = {}, {}, {}

            def load_x(bn, pieces=(4, 4)):
                xt = xs.tile([128, KO, 512], BF16, tag="xT", name=f"x{bn}")
                p0 = 0
                for np_ in pieces:
                    nc.sync.dma_start(
                        xt[:, p0 : p0 + np_, :],
                        xT_r[:, p0 : p0 + np_, bass.ts(bn, 512)],
                    )
                    p0 += np_
                xts[bn] = xt

            def load_kv(bn):
                kT = kv.tile([128, L], BF16, tag="kT", name=f"kT{bn}")
                nc.sync.dma_start(kT[:, : 6 * 128], kTp[bn, :, : 6 * 128])
                nc.sync.dma_start(kT[:, 6 * 128 : PAST], kTp[bn, :, 6 * 128 :])
                va = kv.tile([128, HPC, NCH, HD + 1], BF16, tag="va", name=f"va{bn}")
                nc.sync.dma_start(va[:, :, :PCH, :], vp[bn, :, :, :, :])
                nc.gpsimd.memset(va[:, :, PCH:, HD], 1.0)
                kts[bn], vas[bn] = kT, va

            nc.sync.dma_start(w_s["q"], wq[:, :, :])
            load_x(0, pieces=(4, 2, 1, 1))
            nc.sync.dma_start(w_s["k"], wk[:, :, :])
            load_kv(0)
            nc.sync.dma_start(w_s["v"], wv[:, :, :])
            load_x(1)
            woT_s = persist.tile([128, D], BF16)
            nc.sync.dma_start(woT_s, woT[:, :])

            # ---- constants ----
            identity = const.tile([128, 128], FP32)
            make_identity(nc, identity)
            ident_b = const.tile([128, 128], BF16)
            nc.vector.tensor_copy(ident_b, identity)
            masks = []
            for r in range(4):
                m = const.tile([128, T], FP32, tag=f"mask{r}", name=f"m{r}")
                nc.gpsimd.memset(m, 0.0)
                # keep 0 where query t >= key-token (128r + p), else NEG
                nc.gpsimd.affine_select(
                    out=m, in_=m, compare_op=mybir.AluOpType.is_ge,
                    fill=NEG, base=-128 * r, channel_multiplier=-1,
                    pattern=[[1, T]],
                )
                mb_ = const.tile([128, T], BF16, tag=f"maskb{r}", name=f"mb{r}")
                nc.vector.tensor_copy(mb_, m)
                masks.append(mb_)
            # (no warm-up exp needed: bacc inserts an explicit
            # LoadActFuncSet at the head of the program)

            # ---- persistent SBUF ----
            qT = persist.tile([128, TOK], BF16, tag="qT")
            oT = persist.tile([128, B, QC, 128], BF16, tag="oT")

            # ---- projection chores: <= ~430ns of PE work each ----
            def chores_qk(bn, which):
                """4 chores of 2 ko-steps; evict on the last."""
                box = {}

                def mk(piece):
                    def go():
                        if piece == 0:
                            box["ps"] = flexp.tile(
                                [128, 512], FP32, tag="flex", name=f"p{which}{bn}"
                            )
                        ps = box["ps"]
                        for ko in (2 * piece, 2 * piece + 1):
                            nc.tensor.matmul(
                                ps, lhsT=w_s[which][:, ko, :], rhs=xts[bn][:, ko, :],
                                start=(ko == 0), stop=(ko == KO - 1),
                                skip_group_check=True,
                            )
                        if piece == 3:
                            if which == "q":
                                nc.vector.tensor_copy(qT[:, bass.ts(bn, 512)], ps)
                            else:
                                nc.vector.tensor_copy(kts[bn][:, PAST:], ps)
                    return go

                return [mk(p) for p in range(4)]

            def chores_v(bn):
                """4 chores, one 128-token chunk each (8 matmuls of 128)."""
                def mk(tt):
                    def go():
                        psv = flexp.tile(
                            [128, HPC, HD], FP32, tag="flex", name=f"pv{bn}{tt}"
                        )
                        for ko in range(KO):
                            nc.tensor.matmul(
                                psv, lhsT=xts[bn][:, ko, bass.ts(tt, 128)],
                                rhs=w_s["v"][:, ko, :],
                                start=(ko == 0), stop=(ko == KO - 1),
                                skip_group_check=True,
                            )
                        nc.vector.tensor_copy(vas[bn][:, :, PCH + tt, :HD], psv)
                    return go

                return [mk(tt) for tt in range(QC)]

            osts = {}

            def chores_oproj(bn):
                """8 chores: (tt, nh) matmul + evict; DMA out after nh==1."""
                out_chores = []
                for tt in range(QC):
                    for nh in range(2):
                        def go(tt=tt, nh=nh):
                            ps = flexp.tile(
                                [128, 512], FP32, tag="flex", name=f"po{bn}{tt}{nh}"
                            )
                            nc.tensor.matmul(
                                ps, lhsT=oT[:, bn, tt, :],
                                rhs=woT_s[:, bass.ts(nh, 512)],
                                start=True, stop=True,
                            )
                            if nh == 0:
                                osts[(bn, tt)] = stage.tile(
                                    [128, D], BF16, tag="ost", name=f"ost{bn}{tt}"
                                )
                            ost = osts[(bn, tt)]
                            nc.vector.tensor_copy(ost[:, bass.ts(nh, 512)], ps)
                            if nh == 1:
                                r0 = bn * T + tt * 128
                                nc.sync.dma_start(out[r0 : r0 + 128, :], ost)
                        out_chores.append(go)
                return out_chores

            # one group of score matmuls (+ causal mask) and ONE exp.
            # Masked chunks are packed densely (no garbage columns in the
            # exp): base[cc] is the packed column offset of chunk cc's
            # visible queries [off, 512). The last (2-chunk, 384-column)
            # group lives in the flex bank so the score ring frees early.
            def emit_group(bb, hh, gi):
                hsl = slice(hh * HD, (hh + 1) * HD)
                kT = kts[bb]
                g = groups[gi]
                sc = scp.tile([128, 1536], FP32, tag="sc",
                              name=f"sc{bb}{hh}{gi}")
                if gi == len(groups) - 1:
                    W = 384
                elif gi == len(groups) - 2:
                    W = 1408
                else:
                    W = 512 * len(g)
                bases = {}
                base = 0
                for cc in g:
                    r = cc - PCH
                    off = max(0, r * 128)
                    bases[cc] = (base, off)
                    nc.tensor.matmul(
                        sc[:, base : base + 512 - off],
                        lhsT=kT[hsl, bass.ts(cc, 128)],
                        rhs=qT[hsl, bb * T + off : (bb + 1) * T],
                        start=True, stop=r < 0,
                    )
                    if r >= 0:
                        # only the 128-wide triangle block needs masking
                        nc.tensor.matmul(
                            sc[:, base : base + 128],
                            lhsT=ident_b, rhs=masks[r][:, off : off + 128],
                            start=False, stop=True, skip_group_check=True,
                        )
                    base += 512 - off
                assert base == W, (gi, base, W)
                pT = ptp.tile([128, 1536], BF16, tag="pT", name=f"pT{bb}{hh}{gi}")
                nc.scalar.activation(pT[:, :W], sc[:, :W], Exp)
                return pT, bases

            # ---- attention pair: scores/exp run one group ahead of P.V;
            # the NEXT pair's first group is prefetched before this pair's
            # trailing work so the exp stream never gaps at pair boundaries
            def pair(b, h, osb, chores, ci, last=False, pre=None, nxt=None):
                va = vas[b]
                nxt_pre = None
                acc = accp.tile(
                    [128, QC, HD + 1], FP32, tag="acc", name=f"acc{b}{h}"
                )
                pts = {}

                def finish_qcs(b, h, osb, qlo, qhi):
                    """Last pair only: divide + transpose + o_proj + DMA for
                    query chunks [qlo, qhi) as soon as they stop accumulating.
                    The two chains run in parallel across PE/ScalarE/DVE
                    (the exp stream is done or nearly done here)."""
                    nqc = qhi - qlo
                    r_ = divp.tile([128, nqc], FP32, tag="r", name=f"rL{qlo}")
                    nc.vector.reciprocal(r_, acc[:, qlo:qhi, HD])
                    nc.vector.tensor_tensor(
                        osb[:, qlo:qhi, h, :], acc[:, qlo:qhi, :HD],
                        r_[:, :, None].to_broadcast([128, nqc, HD]), mult,
                    )
                    tps = {}
                    for qc in range(qlo, qhi):
                        # the final call's second transpose goes to the (then
                        # dead) acc bank so the chains don't serialize on the
                        # flex slot; earlier calls must NOT touch the acc bank
                        # (a start=True there wipes the open accumulations)
                        pool, tag = ((accp, "acc") if qc > qlo and qlo >= 2
                                     else (flexp, "flex"))
                        tp = pool.tile([128, 128], BF16, tag=tag,
                                       name=f"tpL{qc}")
                        nc.tensor.transpose(tp, osb[:, qc, :, :], ident_b)
                        tps[qc] = tp
                    for qc in range(qlo, qhi):
                        if qc == qlo:
                            nc.vector.tensor_copy(oT[:, b, qc, :], tps[qc])
                        else:
                            nc.scalar.copy(oT[:, b, qc, :], tps[qc])
                    # 4 o_proj matmuls into 4 DISTINCT psum banks (scA, scB,
                    # then the flex/acc banks that held the dead transposes)
                    # so no matmul WARs on an eviction
                    pss = {}
                    osts = {}
                    for qc in range(qlo, qhi):
                        osts[qc] = stage.tile([128, D], BF16, tag="ost",
                                              name=f"osL{qc}")
                    for nh in range(2):
                        for i, qc in enumerate(range(qlo, qhi)):
                            if nh == 0:
                                ps = scp.tile([128, 3, 512], FP32, tag="sc",
                                              name=f"pL{qc}{nh}")[:, 0, :]
                            else:
                                pool, tag = ((flexp, "flex") if i == 0
                                             else (accp, "acc"))
                                if qlo < 2 and i > 0:
                                    pool, tag = scp, "sc"
                                ps = pool.tile([128, 512], FP32, tag=tag,
                                               name=f"pL{qc}{nh}")
                                if tag == "sc":
                                    ps = ps[:, :512]
                            nc.tensor.matmul(
                                ps, lhsT=oT[:, b, qc, :],
                                rhs=woT_s[:, bass.ts(nh, 512)],
                                start=True, stop=True,
                            )
                            pss[(qc, nh)] = ps
                    # evicts: the early pair leans on ScalarE (its exp stream
                    # is ending), the last pair leans on DVE
                    for i, (qc, nh) in enumerate(
                        (q, n) for n in range(2) for q in range(qlo, qhi)
                    ):
                        ev = osts[qc][:, bass.ts(nh, 512)]
                        act_side = (i != 3) if qlo < 2 else (i >= 2)
                        if act_side:
                            nc.scalar.copy(ev, pss[(qc, nh)])
                        else:
                            nc.vector.tensor_copy(ev, pss[(qc, nh)])
                    for qc in range(qlo, qhi):
                        r0 = b * T + qc * 128
                        if qlo >= 2 and qc == qhi - 1:
                            # last row block: split per half so the first
                            # half's transfer overlaps the second's eviction
                            for nh in range(2):
                                nc.sync.dma_start(
                                    out[r0 : r0 + 128, bass.ts(nh, 512)],
                                    osts[qc][:, bass.ts(nh, 512)],
                                )
                        else:
                            nc.sync.dma_start(out[r0 : r0 + 128, :], osts[qc])

                def pv(gi):
                    pT, bases = pts[gi]
                    for cc in groups[gi]:
                        base, off = bases[cc]
                        for qc in range(max(0, cc - PCH), QC):
                            c0 = base + qc * 128 - off
                            # one start/stop per psum bank per pair: start
                            # marks the WHOLE 2KB bank pending-zero, so each
                            # region's first write overwrites (self-zeroing)
                            nc.tensor.matmul(
                                acc[:, qc, :],
                                lhsT=pT[:, c0 : c0 + 128],
                                rhs=va[:, h, cc, :],
                                start=(cc == 0 and qc == 0),
                                stop=(cc == NCH - 1 and qc == QC - 1),
                                skip_group_check=True,
                            )

                npre = len(pre) if pre is not None else 0
                for gi in range(len(groups)):
                    if gi < npre:
                        pts[gi] = pre[gi]
                    else:
                        pts[gi] = emit_group(b, h, gi)
                    if gi == len(groups) - 1 and nxt is not None:
                        nxt_pre = (emit_group(nxt[0], nxt[1], 0),)
                    if gi > 0:
                        pv(gi - 1)
                    if ci < len(chores) and gi >= npre:
                        chores[ci]()
                        ci += 1
                    if last and gi == len(groups) - 1:
                        # qc 0/1 stopped at chunks 28/29 (group 9): finish
                        # them while ScalarE still runs the last exps
                        finish_qcs(b, h, osb, 0, 2)
                pv(len(groups) - 1)
                if ci < len(chores):
                    chores[ci]()
                    ci += 1

                if last:
                    finish_qcs(b, h, osb, 2, 4)
                else:
                    # divide by the softmax denominator (column 64 of acc)
                    r_ = divp.tile([128, QC], FP32, tag="r", name=f"r{b}{h}")
                    nc.vector.reciprocal(r_, acc[:, :, HD])
                    nc.vector.tensor_tensor(
                        osb[:, :, h, :], acc[:, :, :HD],
                        r_[:, :, None].to_broadcast([128, QC, HD]), mult,
                    )
                return ci, nxt_pre

            def finish_batch(b, osb):
                # o [q, ch] -> oT [ch, q] per query chunk (both heads at once)
                tp = accp.tile([128, QC, 128], BF16, tag="acc", name=f"tp{b}")
                for qc in range(QC):
                    nc.tensor.transpose(tp[:, qc, :], osb[:, qc, :, :], ident_b)
                nc.vector.tensor_copy(oT[:, b], tp)

            # ---- prologue: only batch-0 q projection (ScalarE idle) ----
            ps_q = flexp.tile([128, 512], FP32, tag="flex", name="psq0")
            for ko in range(KO):
                nc.tensor.matmul(
                    ps_q, lhsT=w_s["q"][:, ko, :], rhs=xts[0][:, ko, :],
                    start=(ko == 0), stop=(ko == KO - 1), skip_group_check=True,
                )
            nc.vector.tensor_copy(qT[:, :512], ps_q)

            # ---- main loop ----
            pre = None
            for b in range(B):
                if b + 1 < B:
                    load_kv(b + 1)
                chores = []
                if b == 0:
                    chores += chores_qk(0, "k") + chores_v(0)
                else:
                    chores += chores_oproj(b - 1)
                if b + 1 < B:
                    chores += chores_qk(b + 1, "q") + chores_qk(b + 1, "k")
                    chores += chores_v(b + 1)
                osb = osbp.tile([128, QC, HPC, HD], BF16, tag="osb", name=f"osb{b}")
                ci, pre = pair(b, 0, osb, chores, 0, pre=pre, nxt=(b, 1))
                if b + 2 < B:
                    load_x(b + 2)
                nxt = (b + 1, 0) if b + 1 < B else None
                ci, pre = pair(
                    b, 1, osb, chores, ci, last=(b == B - 1), pre=pre, nxt=nxt
                )
                assert ci >= len(chores), (b, ci, len(chores))
                if b < B - 1:
                    finish_batch(b, osb)

    nc.compile()
    return nc


def _pack_v(v):
    """[B, HPC, PAST, HD] -> [B, 128, HPC, PCH, HD+1] bf16, ones in col HD."""
    o = np.empty((B, 128, HPC, PCH, HD + 1), np.float32)
    o[..., :HD] = v.reshape(B, HPC, PCH, 128, HD).transpose(0, 3, 1, 2, 4)
    o[..., HD] = 1.0
    return np.ascontiguousarray(o.astype(BF))


def _prep(x, k_prev, v_prev, Wq, Wk, Wv, Wo):
    """Host-side shard + layout marshalling (bf16, C-contiguous)."""
    f = np.float32
    x2 = np.asarray(x, f).reshape(TOK, D)
    xT = np.ascontiguousarray(x2.T.astype(BF))
    k_prev = np.asarray(k_prev, f)
    v_prev = np.asarray(v_prev, f)
    Wq, Wk, Wv, Wo = (np.asarray(w, f) for w in (Wq, Wk, Wv, Wo))

    def swz(wT):
        # [D, 128] -> [p, ko, m] with row (ko*128+p) -> [p, ko, :]
        return np.ascontiguousarray(
            wT.reshape(KO, 128, 128).transpose(1, 0, 2).astype(BF)
        )

    in_maps = []
    for c in range(NCORES):
        rows = slice(128 * c, 128 * (c + 1))
        hsl = slice(HPC * c, HPC * (c + 1))
        in_maps.append(
            {
                "xT": xT,
                "wq": swz((Wq[rows, :] * SCALE).T),
                "wk": swz(Wk[rows, :].T),
                "wv": swz(Wv[rows, :].T),
                "woT": np.ascontiguousarray(Wo[:, rows].T.astype(BF)),
                "kTp": np.ascontiguousarray(
                    k_prev[:, hsl, :, :].transpose(0, 1, 3, 2).astype(BF)
                ).reshape(B, 128, PAST),
                "vp": _pack_v(v_prev[:, hsl, :, :]),
            }
        )
    return in_maps


def kernel(x, k_prev, v_prev, Wq, Wk, Wv, Wo):
    if "nc" not in _cache:
        _cache["nc"] = _build()
    nc = _cache["nc"]
    in_maps = _prep(x, k_prev, v_prev, Wq, Wk, Wv, Wo)
    res = run_bass_kernel_spmd(nc, in_maps, core_ids=list(range(NCORES)))
    acc = np.zeros((TOK, D), np.float64)
    for r in res.results:
        acc += np.asarray(r["out"], np.float32)
    return acc.astype(np.float32).reshape(B, T, D)


# revision 50
# speedup vs baseline: 1.3167x; 1.0009x over previous
"""Multi-head attention with KV cache, sharded over 8 NeuronCores by head.

Problem (hardcoded shapes):
  x       [4, 512, 1024]      hidden states (B, T, D)
  k_prev  [4, 16, 3584, 64]   KV cache (B, H, PAST, HD)
  v_prev  [4, 16, 3584, 64]
  Wq/Wk/Wv/Wo [1024, 1024]    projection weights (torch Linear: y = x @ W.T)

Sharding: 16 heads / 8 cores = 2 heads per core; each core computes its
2 heads' q/k/v projections, attention, and a column-parallel o_proj
partial [2048, 1024]; the host sums the 8 partials.

Device algorithm (all matmul operands bf16, fp32 PSUM accumulate;
measured end-to-end rel err ~1e-3 vs the 2e-2 gate):
  - q^T/k^T = W_slice @ x^T (contract D over 8 ko-steps); v computed
    directly in [token, channel] orientation so it lands in the
    [key-partition, head-dim] layout attention needs (no PE transposes).
  - scores^T[key, q] per 128-key chunk (K=HD=64 on partitions); causal
    mask on the 4 newest chunks added via identity @ mask matmul; exp on
    ScalarE (groups of 3 chunks = 3 PSUM banks, double buffered).
  - P.V in [query-partition, head-dim] orientation: out[q, hd] += over
    key chunks with lhsT = pT chunk [keys, q], rhs = [v | 1] [keys, 65]
    so column 64 accumulates the softmax denominator. 65-wide outputs
    cost 65 PE cycles/matmul vs 512 for the transposed orientation.
  - divide by denominator on DVE (free-dim broadcast), PE-transpose o to
    [channel, token] for o_proj, o_proj column-parallel, bf16 out.
  Scheduling: exp is the critical path (~123us on ScalarE). The P.V
  matmuls trail the score matmuls by one group so PE never blocks the
  exp stream, and projection/o_proj work is split into <=450ns "chores"
  threaded one-per-group between attention groups.
"""

import numpy as np
import ml_dtypes

import concourse.bass as bass
import concourse.mybir as mybir
import concourse.tile as tile
from concourse import bacc
from concourse.bass_utils import run_bass_kernel_spmd
from concourse.masks import make_identity

B, T, D = 4, 512, 1024
H, HD = 16, 64
PAST = 3584
L = PAST + T            # 4096 == MAX_CACHE, nothing is trimmed
SCALE = 1.0 / np.sqrt(HD).astype(np.float32)
NCORES = 8
HPC = H // NCORES       # heads per core = 2
TOK = B * T             # 2048
NCH = L // 128          # 32 key chunks per (b, h)
PCH = PAST // 128       # 28 chunks from the cache
QC = T // 128           # 4 query chunks of 128
KO = D // 128           # 8 contraction steps for projections
FP32 = mybir.dt.float32
BF16 = mybir.dt.bfloat16
BF = ml_dtypes.bfloat16
NEG = -1.0e30

_cache = {}


def _build():
    nc = bacc.Bacc(None, target_bir_lowering=False)

    xT = nc.dram_tensor("xT", [D, TOK], BF16, kind="ExternalInput")
    # weights pre-swizzled on host to [p, ko, m] so DMA rows are 2KB
    wq = nc.dram_tensor("wq", [128, KO, 128], BF16, kind="ExternalInput")
    wk = nc.dram_tensor("wk", [128, KO, 128], BF16, kind="ExternalInput")
    wv = nc.dram_tensor("wv", [128, KO, 128], BF16, kind="ExternalInput")
    woT = nc.dram_tensor("woT", [128, D], BF16, kind="ExternalInput")
    kTp = nc.dram_tensor("kTp", [B, 128, PAST], BF16, kind="ExternalInput")
    vp = nc.dram_tensor("vp", [B, 128, HPC, PCH, HD + 1], BF16, kind="ExternalInput")
    out = nc.dram_tensor("out", [TOK, D], BF16, kind="ExternalOutput")

    Exp = mybir.ActivationFunctionType.Exp
    mult = mybir.AluOpType.mult

    # key-chunk groups: a scores psum tile holds up to 3 chunks (3 banks)
    groups = [list(range(s, min(s + 3, NCH))) for s in range(0, NCH, 3)]
    xT_r = xT.rearrange("(ko p) t -> p ko t", p=128)

    with tile.TileContext(nc) as tc:
        with (
            tc.tile_pool(name="const", bufs=1) as const,
            tc.tile_pool(name="persist", bufs=1) as persist,
            tc.tile_pool(name="kv", bufs=2) as kv,
            tc.tile_pool(name="pt", bufs=4) as ptp,
            tc.tile_pool(name="div", bufs=2) as divp,
            tc.tile_pool(name="osb", bufs=2) as osbp,
            tc.tile_pool(name="stage", bufs=4) as stage,
            tc.tile_pool(name="xw", bufs=1) as xw,
            tc.tile_pool(name="xs", bufs=2) as xs,
            tc.tile_pool(name="acc_ps", bufs=1, space="PSUM") as accp,
            tc.tile_pool(name="flex_ps", bufs=1, space="PSUM") as flexp,
            tc.tile_pool(name="sc_ps", bufs=2, space="PSUM") as scp,
        ):
            # ---- PE p-state warm-up: keep PE busy from t~0 so the clock
            # ramps to full speed before the first projection; operands are
            # uninitialized (outputs land in a psum bank nobody reads) ----
            dmy = const.tile([128, 128], BF16, tag="dmy")
            nc.gpsimd.memset(dmy, 0.0)
            dps = flexp.tile([128, 512], FP32, tag="flex", name="dps")
            for i in range(40):
                nc.tensor.matmul(
                    dps[:, :128], lhsT=dmy, rhs=dmy,
                    start=True, stop=True, skip_group_check=True,
                )

            # ---- DMAs next, ordered by first use ----
            w_s = {}
            for nm in ("q", "k", "v"):
                w_s[nm] = xw.tile([128, KO, 128], BF16, tag=f"w{nm}", name=f"w{nm}")
            xts, kts, vas = {}, {}, {}

            def load_x(bn, pieces=(4, 4)):
                xt = xs.tile([128, KO, 512], BF16, tag="xT", name=f"x{bn}")
                p0 = 0
                for np_ in pieces:
                    nc.sync.dma_start(
                        xt[:, p0 : p0 + np_, :],
                        xT_r[:, p0 : p0 + np_, bass.ts(bn, 512)],
                    )
                    p0 += np_
                xts[bn] = xt

            def load_kv(bn):
                kT = kv.tile([128, L], BF16, tag="kT", name=f"kT{bn}")
                nc.sync.dma_start(kT[:, : 6 * 128], kTp[bn, :, : 6 * 128])
                nc.sync.dma_start(kT[:, 6 * 128 : PAST], kTp[bn, :, 6 * 128 :])
                va = kv.tile([128, HPC, NCH, HD + 1], BF16, tag="va", name=f"va{bn}")
                nc.sync.dma_start(va[:, :, :PCH, :], vp[bn, :, :, :, :])
                nc.gpsimd.memset(va[:, :, PCH:, HD], 1.0)
                kts[bn], vas[bn] = kT, va

            nc.sync.dma_start(w_s["q"], wq[:, :, :])
            load_x(0, pieces=(4, 2, 1, 1))
            nc.sync.dma_start(w_s["k"], wk[:, :, :])
            load_kv(0)
            nc.sync.dma_start(w_s["v"], wv[:, :, :])
            load_x(1)
            woT_s = persist.tile([128, D], BF16)
            nc.sync.dma_start(woT_s, woT[:, :])

            # ---- constants ----
            identity = const.tile([128, 128], FP32)
            make_identity(nc, identity)
            ident_b = const.tile([128, 128], BF16)
            nc.vector.tensor_copy(ident_b, identity)
            # (mask building is deferred until after the batch-0 q
            # projection so its DVE copies don't queue ahead of the
            # projection evict; no warm-up exp needed: bacc inserts an
            # explicit LoadActFuncSet at the head of the program)
            masks = []

            # ---- persistent SBUF ----
            qT = persist.tile([128, TOK], BF16, tag="qT")
            oT = persist.tile([128, B, QC, 128], BF16, tag="oT")

            # ---- projection chores: <= ~430ns of PE work each ----
            def chores_qk(bn, which):
                """4 chores of 2 ko-steps; evict on the last."""
                box = {}

                def mk(piece):
                    def go():
                        if piece == 0:
                            box["ps"] = flexp.tile(
                                [128, 512], FP32, tag="flex", name=f"p{which}{bn}"
                            )
                        ps = box["ps"]
                        for ko in (2 * piece, 2 * piece + 1):
                            nc.tensor.matmul(
                                ps, lhsT=w_s[which][:, ko, :], rhs=xts[bn][:, ko, :],
                                start=(ko == 0), stop=(ko == KO - 1),
                                skip_group_check=True,
                            )
                        if piece == 3:
                            if which == "q":
                                nc.vector.tensor_copy(qT[:, bass.ts(bn, 512)], ps)
                            else:
                                nc.vector.tensor_copy(kts[bn][:, PAST:], ps)
                    return go

                return [mk(p) for p in range(4)]

            def chores_v(bn):
                """4 chores, one 128-token chunk each (8 matmuls of 128)."""
                def mk(tt):
                    def go():
                        psv = flexp.tile(
                            [128, HPC, HD], FP32, tag="flex", name=f"pv{bn}{tt}"
                        )
                        for ko in range(KO):
                            nc.tensor.matmul(
                                psv, lhsT=xts[bn][:, ko, bass.ts(tt, 128)],
                                rhs=w_s["v"][:, ko, :],
                                start=(ko == 0), stop=(ko == KO - 1),
                                skip_group_check=True,
                            )
                        nc.vector.tensor_copy(vas[bn][:, :, PCH + tt, :HD], psv)
                    return go

                return [mk(tt) for tt in range(QC)]

            osts = {}

            def chores_oproj(bn):
                """8 chores: (tt, nh) matmul + evict; DMA out after nh==1."""
                out_chores = []
                for tt in range(QC):
                    for nh in range(2):
                        def go(tt=tt, nh=nh):
                            ps = flexp.tile(
                                [128, 512], FP32, tag="flex", name=f"po{bn}{tt}{nh}"
                            )
                            nc.tensor.matmul(
                                ps, lhsT=oT[:, bn, tt, :],
                                rhs=woT_s[:, bass.ts(nh, 512)],
                                start=True, stop=True,
                            )
                            if nh == 0:
                                osts[(bn, tt)] = stage.tile(
                                    [128, D], BF16, tag="ost", name=f"ost{bn}{tt}"
                                )
                            ost = osts[(bn, tt)]
                            nc.vector.tensor_copy(ost[:, bass.ts(nh, 512)], ps)
                            if nh == 1:
                                r0 = bn * T + tt * 128
                                nc.sync.dma_start(out[r0 : r0 + 128, :], ost)
                        out_chores.append(go)
                return out_chores

            # one group of score matmuls (+ causal mask) and ONE exp.
            # Masked chunks are packed densely (no garbage columns in the
            # exp): base[cc] is the packed column offset of chunk cc's
            # visible queries [off, 512). The last (2-chunk, 384-column)
            # group lives in the flex bank so the score ring frees early.
            def emit_group(bb, hh, gi):
                hsl = slice(hh * HD, (hh + 1) * HD)
                kT = kts[bb]
                g = groups[gi]
                sc = scp.tile([128, 1536], FP32, tag="sc",
                              name=f"sc{bb}{hh}{gi}")
                if gi == len(groups) - 1:
                    W = 384
                elif gi == len(groups) - 2:
                    W = 1408
                else:
                    W = 512 * len(g)
                bases = {}
                base = 0
                for cc in g:
                    r = cc - PCH
                    off = max(0, r * 128)
                    bases[cc] = (base, off)
                    nc.tensor.matmul(
                        sc[:, base : base + 512 - off],
                        lhsT=kT[hsl, bass.ts(cc, 128)],
                        rhs=qT[hsl, bb * T + off : (bb + 1) * T],
                        start=True, stop=r < 0,
                    )
                    if r >= 0:
                        # only the 128-wide triangle block needs masking
                        nc.tensor.matmul(
                            sc[:, base : base + 128],
                            lhsT=ident_b, rhs=masks[r][:, off : off + 128],
                            start=False, stop=True, skip_group_check=True,
                        )
                    base += 512 - off
                assert base == W, (gi, base, W)
                pT = ptp.tile([128, 1536], BF16, tag="pT", name=f"pT{bb}{hh}{gi}")
                nc.scalar.activation(pT[:, :W], sc[:, :W], Exp)
                return pT, bases

            # ---- attention pair: scores/exp run one group ahead of P.V;
            # the NEXT pair's first group is prefetched before this pair's
            # trailing work so the exp stream never gaps at pair boundaries
            def pair(b, h, osb, chores, ci, last=False, pre=None, nxt=None):
                va = vas[b]
                nxt_pre = None
                acc = accp.tile(
                    [128, QC, HD + 1], FP32, tag="acc", name=f"acc{b}{h}"
                )
                pts = {}

                def finish_qcs(b, h, osb, qlo, qhi):
                    """Last pair only: divide + transpose + o_proj + DMA for
                    query chunks [qlo, qhi) as soon as they stop accumulating.
                    The two chains run in parallel across PE/ScalarE/DVE
                    (the exp stream is done or nearly done here)."""
                    nqc = qhi - qlo
                    r_ = divp.tile([128, nqc], FP32, tag="r", name=f"rL{qlo}")
                    nc.vector.reciprocal(r_, acc[:, qlo:qhi, HD])
                    nc.vector.tensor_tensor(
                        osb[:, qlo:qhi, h, :], acc[:, qlo:qhi, :HD],
                        r_[:, :, None].to_broadcast([128, nqc, HD]), mult,
                    )
                    tps = {}
                    for qc in range(qlo, qhi):
                        # the final call's second transpose goes to the (then
                        # dead) acc bank so the chains don't serialize on the
                        # flex slot; earlier calls must NOT touch the acc bank
                        # (a start=True there wipes the open accumulations)
                        pool, tag = ((accp, "acc") if qc > qlo and qlo >= 2
                                     else (flexp, "flex"))
                        tp = pool.tile([128, 128], BF16, tag=tag,
                                       name=f"tpL{qc}")
                        nc.tensor.transpose(tp, osb[:, qc, :, :], ident_b)
                        tps[qc] = tp
                    for qc in range(qlo, qhi):
                        if qc == qlo:
                            nc.vector.tensor_copy(oT[:, b, qc, :], tps[qc])
                        else:
                            nc.scalar.copy(oT[:, b, qc, :], tps[qc])
                    # 4 o_proj matmuls into 4 DISTINCT psum banks (scA, scB,
                    # then the flex/acc banks that held the dead transposes)
                    # so no matmul WARs on an eviction
                    pss = {}
                    osts = {}
                    for qc in range(qlo, qhi):
                        osts[qc] = stage.tile([128, D], BF16, tag="ost",
                                              name=f"osL{qc}")
                    for nh in range(2):
                        for i, qc in enumerate(range(qlo, qhi)):
                            if nh == 0:
                                ps = scp.tile([128, 3, 512], FP32, tag="sc",
                                              name=f"pL{qc}{nh}")[:, 0, :]
                            else:
                                pool, tag = ((flexp, "flex") if i == 0
                                             else (accp, "acc"))
                                if qlo < 2 and i > 0:
                                    pool, tag = scp, "sc"
                                ps = pool.tile([128, 512], FP32, tag=tag,
                                               name=f"pL{qc}{nh}")
                                if tag == "sc":
                                    ps = ps[:, :512]
                            nc.tensor.matmul(
                                ps, lhsT=oT[:, b, qc, :],
                                rhs=woT_s[:, bass.ts(nh, 512)],
                                start=True, stop=True,
                            )
                            pss[(qc, nh)] = ps
                    # evicts: the early pair leans on ScalarE (its exp stream
                    # is ending), the last pair leans on DVE
                    for i, (qc, nh) in enumerate(
                        (q, n) for n in range(2) for q in range(qlo, qhi)
                    ):
                        ev = osts[qc][:, bass.ts(nh, 512)]
                        act_side = (i != 3) if qlo < 2 else (i >= 2)
                        if act_side:
                            nc.scalar.copy(ev, pss[(qc, nh)])
                        else:
                            nc.vector.tensor_copy(ev, pss[(qc, nh)])
                    for qc in range(qlo, qhi):
                        r0 = b * T + qc * 128
                        if qlo >= 2 and qc == qhi - 1:
                            # last row block: split per half so the first
                            # half's transfer overlaps the second's eviction
                            for nh in range(2):
                                nc.sync.dma_start(
                                    out[r0 : r0 + 128, bass.ts(nh, 512)],
                                    osts[qc][:, bass.ts(nh, 512)],
                                )
                        else:
                            nc.sync.dma_start(out[r0 : r0 + 128, :], osts[qc])

                def pv(gi):
                    pT, bases = pts[gi]
                    for cc in groups[gi]:
                        base, off = bases[cc]
                        for qc in range(max(0, cc - PCH), QC):
                            c0 = base + qc * 128 - off
                            # one start/stop per psum bank per pair: start
                            # marks the WHOLE 2KB bank pending-zero, so each
                            # region's first write overwrites (self-zeroing)
                            nc.tensor.matmul(
                                acc[:, qc, :],
                                lhsT=pT[:, c0 : c0 + 128],
                                rhs=va[:, h, cc, :],
                                start=(cc == 0 and qc == 0),
                                stop=(cc == NCH - 1 and qc == QC - 1),
                                skip_group_check=True,
                            )

                npre = len(pre) if pre is not None else 0
                for gi in range(len(groups)):
                    if gi < npre:
                        pts[gi] = pre[gi]
                    else:
                        pts[gi] = emit_group(b, h, gi)
                    if gi == len(groups) - 1 and nxt is not None:
                        nxt_pre = (emit_group(nxt[0], nxt[1], 0),)
                    if gi > 0:
                        pv(gi - 1)
                    if ci < len(chores) and gi >= npre:
                        chores[ci]()
                        ci += 1
                    if last and gi == len(groups) - 1:
                        # qc 0/1 stopped at chunks 28/29 (group 9): finish
                        # them while ScalarE still runs the last exps
                        finish_qcs(b, h, osb, 0, 2)
                pv(len(groups) - 1)
                if ci < len(chores):
                    chores[ci]()
                    ci += 1

                if last:
                    finish_qcs(b, h, osb, 2, 4)
                else:
                    # divide by the softmax denominator (column 64 of acc)
                    r_ = divp.tile([128, QC], FP32, tag="r", name=f"r{b}{h}")
                    nc.vector.reciprocal(r_, acc[:, :, HD])
                    nc.vector.tensor_tensor(
                        osb[:, :, h, :], acc[:, :, :HD],
                        r_[:, :, None].to_broadcast([128, QC, HD]), mult,
                    )
                return ci, nxt_pre

            def finish_batch(b, osb):
                # o [q, ch] -> oT [ch, q] per query chunk (both heads at once)
                tp = accp.tile([128, QC, 128], BF16, tag="acc", name=f"tp{b}")
                for qc in range(QC):
                    nc.tensor.transpose(tp[:, qc, :], osb[:, qc, :, :], ident_b)
                nc.vector.tensor_copy(oT[:, b], tp)

            # ---- prologue: only batch-0 q projection (ScalarE idle) ----
            ps_q = flexp.tile([128, 512], FP32, tag="flex", name="psq0")
            for ko in range(KO):
                nc.tensor.matmul(
                    ps_q, lhsT=w_s["q"][:, ko, :], rhs=xts[0][:, ko, :],
                    start=(ko == 0), stop=(ko == KO - 1), skip_group_check=True,
                )
            # evict on the (idle) Scalar engine: DVE would serialize this
            # behind the mask copies
            nc.scalar.copy(qT[:, :512], ps_q)
            for r in range(4):
                m = const.tile([128, T], FP32, tag=f"mask{r}", name=f"m{r}")
                nc.gpsimd.memset(m, 0.0)
                # keep 0 where query t >= key-token (128r + p), else NEG
                nc.gpsimd.affine_select(
                    out=m, in_=m, compare_op=mybir.AluOpType.is_ge,
                    fill=NEG, base=-128 * r, channel_multiplier=-1,
                    pattern=[[1, T]],
                )
                mb_ = const.tile([128, T], BF16, tag=f"maskb{r}", name=f"mb{r}")
                nc.vector.tensor_copy(mb_, m)
                masks.append(mb_)

            # ---- main loop ----
            pre = None
            for b in range(B):
                if b + 1 < B:
                    load_kv(b + 1)
                chores = []
                if b == 0:
                    chores += chores_qk(0, "k") + chores_v(0)
                else:
                    chores += chores_oproj(b - 1)
                if b + 1 < B:
                    chores += chores_qk(b + 1, "q") + chores_qk(b + 1, "k")
                    chores += chores_v(b + 1)
                osb = osbp.tile([128, QC, HPC, HD], BF16, tag="osb", name=f"osb{b}")
                ci, pre = pair(b, 0, osb, chores, 0, pre=pre, nxt=(b, 1))
                if b + 2 < B:
                    load_x(b + 2)
                nxt = (b + 1, 0) if b + 1 < B else None
                ci, pre = pair(
                    b, 1, osb, chores, ci, last=(b == B - 1), pre=pre, nxt=nxt
                )
                assert ci >= len(chores), (b, ci, len(chores))
                if b < B - 1:
                    finish_batch(b, osb)

    nc.compile()
    return nc


def _pack_v(v):
    """[B, HPC, PAST, HD] -> [B, 128, HPC, PCH, HD+1] bf16, ones in col HD."""
    o = np.empty((B, 128, HPC, PCH, HD + 1), np.float32)
    o[..., :HD] = v.reshape(B, HPC, PCH, 128, HD).transpose(0, 3, 1, 2, 4)
    o[..., HD] = 1.0
    return np.ascontiguousarray(o.astype(BF))


def _prep(x, k_prev, v_prev, Wq, Wk, Wv, Wo):
    """Host-side shard + layout marshalling (bf16, C-contiguous)."""
    f = np.float32
    x2 = np.asarray(x, f).reshape(TOK, D)
    xT = np.ascontiguousarray(x2.T.astype(BF))
    k_prev = np.asarray(k_prev, f)
    v_prev = np.asarray(v_prev, f)
    Wq, Wk, Wv, Wo = (np.asarray(w, f) for w in (Wq, Wk, Wv, Wo))

    def swz(wT):
        # [D, 128] -> [p, ko, m] with row (ko*128+p) -> [p, ko, :]
        return np.ascontiguousarray(
            wT.reshape(KO, 128, 128).transpose(1, 0, 2).astype(BF)
        )

    in_maps = []
    for c in range(NCORES):
        rows = slice(128 * c, 128 * (c + 1))
        hsl = slice(HPC * c, HPC * (c + 1))
        in_maps.append(
            {
                "xT": xT,
                "wq": swz((Wq[rows, :] * SCALE).T),
                "wk": swz(Wk[rows, :].T),
                "wv": swz(Wv[rows, :].T),
                "woT": np.ascontiguousarray(Wo[:, rows].T.astype(BF)),
                "kTp": np.ascontiguousarray(
                    k_prev[:, hsl, :, :].transpose(0, 1, 3, 2).astype(BF)
                ).reshape(B, 128, PAST),
                "vp": _pack_v(v_prev[:, hsl, :, :]),
            }
        )
    return in_maps


def kernel(x, k_prev, v_prev, Wq, Wk, Wv, Wo):
    if "nc" not in _cache:
        _cache["nc"] = _build()
    nc = _cache["nc"]
    in_maps = _prep(x, k_prev, v_prev, Wq, Wk, Wv, Wo)
    res = run_bass_kernel_spmd(nc, in_maps, core_ids=list(range(NCORES)))
    acc = np.zeros((TOK, D), np.float64)
    for r in res.results:
        acc += np.asarray(r["out"], np.float32)
    return acc.astype(np.float32).reshape(B, T, D)


# revision 53
# speedup vs baseline: 1.3208x; 1.0031x over previous
"""Multi-head attention with KV cache, sharded over 8 NeuronCores by head.

Problem (hardcoded shapes):
  x       [4, 512, 1024]      hidden states (B, T, D)
  k_prev  [4, 16, 3584, 64]   KV cache (B, H, PAST, HD)
  v_prev  [4, 16, 3584, 64]
  Wq/Wk/Wv/Wo [1024, 1024]    projection weights (torch Linear: y = x @ W.T)

Sharding: 16 heads / 8 cores = 2 heads per core; each core computes its
2 heads' q/k/v projections, attention, and a column-parallel o_proj
partial [2048, 1024]; the host sums the 8 partials.

Device algorithm (all matmul operands bf16, fp32 PSUM accumulate;
measured end-to-end rel err ~1e-3 vs the 2e-2 gate):
  - q^T/k^T = W_slice @ x^T (contract D over 8 ko-steps); v computed
    directly in [token, channel] orientation so it lands in the
    [key-partition, head-dim] layout attention needs (no PE transposes).
  - scores^T[key, q] per 128-key chunk (K=HD=64 on partitions); causal
    mask on the 4 newest chunks added via identity @ mask matmul; exp on
    ScalarE (groups of 3 chunks = 3 PSUM banks, double buffered).
  - P.V in [query-partition, head-dim] orientation: out[q, hd] += over
    key chunks with lhsT = pT chunk [keys, q], rhs = [v | 1] [keys, 65]
    so column 64 accumulates the softmax denominator. 65-wide outputs
    cost 65 PE cycles/matmul vs 512 for the transposed orientation.
  - divide by denominator on DVE (free-dim broadcast), PE-transpose o to
    [channel, token] for o_proj, o_proj column-parallel, bf16 out.
  Scheduling: exp is the critical path (~123us on ScalarE). The P.V
  matmuls trail the score matmuls by one group so PE never blocks the
  exp stream, and projection/o_proj work is split into <=450ns "chores"
  threaded one-per-group between attention groups.
"""

import numpy as np
import ml_dtypes

import concourse.bass as bass
import concourse.mybir as mybir
import concourse.tile as tile
from concourse import bacc
from concourse.bass_utils import run_bass_kernel_spmd
from concourse.masks import make_identity

B, T, D = 4, 512, 1024
H, HD = 16, 64
PAST = 3584
L = PAST + T            # 4096 == MAX_CACHE, nothing is trimmed
SCALE = 1.0 / np.sqrt(HD).astype(np.float32)
NCORES = 8
HPC = H // NCORES       # heads per core = 2
TOK = B * T             # 2048
NCH = L // 128          # 32 key chunks per (b, h)
PCH = PAST // 128       # 28 chunks from the cache
QC = T // 128           # 4 query chunks of 128
KO = D // 128           # 8 contraction steps for projections
FP32 = mybir.dt.float32
BF16 = mybir.dt.bfloat16
BF = ml_dtypes.bfloat16
NEG = -1.0e30

_cache = {}


def _build():
    nc = bacc.Bacc(None, target_bir_lowering=False)

    xT = nc.dram_tensor("xT", [D, TOK], BF16, kind="ExternalInput")
    # weights pre-swizzled on host to [p, ko, m] so DMA rows are 2KB
    wq = nc.dram_tensor("wq", [128, KO, 128], BF16, kind="ExternalInput")
    wk = nc.dram_tensor("wk", [128, KO, 128], BF16, kind="ExternalInput")
    wv = nc.dram_tensor("wv", [128, KO, 128], BF16, kind="ExternalInput")
    woT = nc.dram_tensor("woT", [128, D], BF16, kind="ExternalInput")
    kTp = nc.dram_tensor("kTp", [B, 128, PAST], BF16, kind="ExternalInput")
    vp = nc.dram_tensor("vp", [B, 128, HPC, PCH, HD + 1], BF16, kind="ExternalInput")
    out = nc.dram_tensor("out", [TOK, D], BF16, kind="ExternalOutput")

    Exp = mybir.ActivationFunctionType.Exp
    mult = mybir.AluOpType.mult

    # key-chunk groups: a scores psum tile holds up to 3 chunks (3 banks)
    groups = [list(range(s, min(s + 3, NCH))) for s in range(0, NCH, 3)]
    xT_r = xT.rearrange("(ko p) t -> p ko t", p=128)

    with tile.TileContext(nc) as tc:
        with (
            tc.tile_pool(name="const", bufs=1) as const,
            tc.tile_pool(name="persist", bufs=1) as persist,
            tc.tile_pool(name="kv", bufs=2) as kv,
            tc.tile_pool(name="pt", bufs=4) as ptp,
            tc.tile_pool(name="div", bufs=2) as divp,
            tc.tile_pool(name="osb", bufs=2) as osbp,
            tc.tile_pool(name="stage", bufs=4) as stage,
            tc.tile_pool(name="xw", bufs=1) as xw,
            tc.tile_pool(name="xs", bufs=2) as xs,
            tc.tile_pool(name="acc_ps", bufs=1, space="PSUM") as accp,
            tc.tile_pool(name="flex_ps", bufs=1, space="PSUM") as flexp,
            tc.tile_pool(name="sc_ps", bufs=2, space="PSUM") as scp,
        ):
            # ---- PE p-state warm-up: keep PE busy from t~0 so the clock
            # ramps to full speed before the first projection; operands are
            # uninitialized (outputs land in a psum bank nobody reads) ----
            dmy = const.tile([128, 128], BF16, tag="dmy")
            nc.gpsimd.memset(dmy, 0.0)
            dps = flexp.tile([128, 512], FP32, tag="flex", name="dps")
            for i in range(40):
                nc.tensor.matmul(
                    dps[:, :128], lhsT=dmy, rhs=dmy,
                    start=True, stop=True, skip_group_check=True,
                )

            # ---- DMAs next, ordered by first use ----
            w_s = {}
            for nm in ("q", "k", "v"):
                w_s[nm] = xw.tile([128, KO, 128], BF16, tag=f"w{nm}", name=f"w{nm}")
            xts, kts, vas = {}, {}, {}

            def load_x(bn, pieces=(4, 4)):
                xt = xs.tile([128, KO, 512], BF16, tag="xT", name=f"x{bn}")
                p0 = 0
                for np_ in pieces:
                    nc.sync.dma_start(
                        xt[:, p0 : p0 + np_, :],
                        xT_r[:, p0 : p0 + np_, bass.ts(bn, 512)],
                    )
                    p0 += np_
                xts[bn] = xt

            def load_kv(bn, split=False):
                kT = kv.tile([128, L], BF16, tag="kT", name=f"kT{bn}")
                nc.sync.dma_start(kT[:, : 6 * 128], kTp[bn, :, : 6 * 128])
                va = kv.tile([128, HPC, NCH, HD + 1], BF16, tag="va", name=f"va{bn}")
                if split:
                    # first score/P.V groups gate on the early chunks; the
                    # bulk of the cache trails the weight loads
                    nc.sync.dma_start(va[:, :, :6, :], vp[bn, :, :, :6, :])
                    nc.sync.dma_start(w_s["k"], wk[:, :, :])
                    nc.sync.dma_start(kT[:, 6 * 128 : PAST], kTp[bn, :, 6 * 128 :])
                    nc.sync.dma_start(va[:, :, 6:PCH, :], vp[bn, :, :, 6:, :])
                else:
                    nc.sync.dma_start(kT[:, 6 * 128 : PAST], kTp[bn, :, 6 * 128 :])
                    nc.sync.dma_start(va[:, :, :PCH, :], vp[bn, :, :, :, :])
                nc.gpsimd.memset(va[:, :, PCH:, HD], 1.0)
                kts[bn], vas[bn] = kT, va

            nc.sync.dma_start(w_s["q"], wq[:, :, :])
            load_x(0, pieces=(4, 2, 1, 1))
            load_kv(0, split=True)
            nc.sync.dma_start(w_s["v"], wv[:, :, :])
            load_x(1)
            woT_s = persist.tile([128, D], BF16)
            nc.sync.dma_start(woT_s, woT[:, :])

            # ---- constants ----
            identity = const.tile([128, 128], FP32)
            make_identity(nc, identity)
            ident_b = const.tile([128, 128], BF16)
            nc.vector.tensor_copy(ident_b, identity)
            # (mask building is deferred until after the batch-0 q
            # projection so its DVE copies don't queue ahead of the
            # projection evict; no warm-up exp needed: bacc inserts an
            # explicit LoadActFuncSet at the head of the program)
            masks = []

            # ---- persistent SBUF ----
            qT = persist.tile([128, TOK], BF16, tag="qT")
            oT = persist.tile([128, B, QC, 128], BF16, tag="oT")

            # ---- projection chores: <= ~430ns of PE work each ----
            def chores_qk(bn, which):
                """4 chores of 2 ko-steps; evict on the last."""
                box = {}

                def mk(piece):
                    def go():
                        if piece == 0:
                            box["ps"] = flexp.tile(
                                [128, 512], FP32, tag="flex", name=f"p{which}{bn}"
                            )
                        ps = box["ps"]
                        for ko in (2 * piece, 2 * piece + 1):
                            nc.tensor.matmul(
                                ps, lhsT=w_s[which][:, ko, :], rhs=xts[bn][:, ko, :],
                                start=(ko == 0), stop=(ko == KO - 1),
                                skip_group_check=True,
                            )
                        if piece == 3:
                            if which == "q":
                                nc.vector.tensor_copy(qT[:, bass.ts(bn, 512)], ps)
                            else:
                                nc.vector.tensor_copy(kts[bn][:, PAST:], ps)
                    return go

                return [mk(p) for p in range(4)]

            def chores_v(bn):
                """4 chores, one 128-token chunk each (8 matmuls of 128)."""
                def mk(tt):
                    def go():
                        psv = flexp.tile(
                            [128, HPC, HD], FP32, tag="flex", name=f"pv{bn}{tt}"
                        )
                        for ko in range(KO):
                            nc.tensor.matmul(
                                psv, lhsT=xts[bn][:, ko, bass.ts(tt, 128)],
                                rhs=w_s["v"][:, ko, :],
                                start=(ko == 0), stop=(ko == KO - 1),
                                skip_group_check=True,
                            )
                        nc.vector.tensor_copy(vas[bn][:, :, PCH + tt, :HD], psv)
                    return go

                return [mk(tt) for tt in range(QC)]

            osts = {}

            def chores_oproj(bn):
                """8 chores: (tt, nh) matmul + evict; DMA out after nh==1."""
                out_chores = []
                for tt in range(QC):
                    for nh in range(2):
                        def go(tt=tt, nh=nh):
                            ps = flexp.tile(
                                [128, 512], FP32, tag="flex", name=f"po{bn}{tt}{nh}"
                            )
                            nc.tensor.matmul(
                                ps, lhsT=oT[:, bn, tt, :],
                                rhs=woT_s[:, bass.ts(nh, 512)],
                                start=True, stop=True,
                            )
                            if nh == 0:
                                osts[(bn, tt)] = stage.tile(
                                    [128, D], BF16, tag="ost", name=f"ost{bn}{tt}"
                                )
                            ost = osts[(bn, tt)]
                            nc.vector.tensor_copy(ost[:, bass.ts(nh, 512)], ps)
                            if nh == 1:
                                r0 = bn * T + tt * 128
                                nc.sync.dma_start(out[r0 : r0 + 128, :], ost)
                        out_chores.append(go)
                return out_chores

            # one group of score matmuls (+ causal mask) and ONE exp.
            # Masked chunks are packed densely (no garbage columns in the
            # exp): base[cc] is the packed column offset of chunk cc's
            # visible queries [off, 512). The last (2-chunk, 384-column)
            # group lives in the flex bank so the score ring frees early.
            def emit_group(bb, hh, gi):
                hsl = slice(hh * HD, (hh + 1) * HD)
                kT = kts[bb]
                g = groups[gi]
                sc = scp.tile([128, 1536], FP32, tag="sc",
                              name=f"sc{bb}{hh}{gi}")
                if gi == len(groups) - 1:
                    W = 384
                elif gi == len(groups) - 2:
                    W = 1408
                else:
                    W = 512 * len(g)
                bases = {}
                base = 0
                for cc in g:
                    r = cc - PCH
                    off = max(0, r * 128)
                    bases[cc] = (base, off)
                    nc.tensor.matmul(
                        sc[:, base : base + 512 - off],
                        lhsT=kT[hsl, bass.ts(cc, 128)],
                        rhs=qT[hsl, bb * T + off : (bb + 1) * T],
                        start=True, stop=r < 0,
                    )
                    if r >= 0:
                        # only the 128-wide triangle block needs masking
                        nc.tensor.matmul(
                            sc[:, base : base + 128],
                            lhsT=ident_b, rhs=masks[r][:, off : off + 128],
                            start=False, stop=True, skip_group_check=True,
                        )
                    base += 512 - off
                assert base == W, (gi, base, W)
                pT = ptp.tile([128, 1536], BF16, tag="pT", name=f"pT{bb}{hh}{gi}")
                nc.scalar.activation(pT[:, :W], sc[:, :W], Exp)
                return pT, bases

            # ---- attention pair: scores/exp run one group ahead of P.V;
            # the NEXT pair's first group is prefetched before this pair's
            # trailing work so the exp stream never gaps at pair boundaries
            def pair(b, h, osb, chores, ci, last=False, pre=None, nxt=None):
                va = vas[b]
                nxt_pre = None
                acc = accp.tile(
                    [128, QC, HD + 1], FP32, tag="acc", name=f"acc{b}{h}"
                )
                pts = {}

                def finish_qcs(b, h, osb, qlo, qhi):
                    """Last pair only: divide + transpose + o_proj + DMA for
                    query chunks [qlo, qhi) as soon as they stop accumulating.
                    The two chains run in parallel across PE/ScalarE/DVE
                    (the exp stream is done or nearly done here)."""
                    nqc = qhi - qlo
                    r_ = divp.tile([128, nqc], FP32, tag="r", name=f"rL{qlo}")
                    nc.vector.reciprocal(r_, acc[:, qlo:qhi, HD])
                    nc.vector.tensor_tensor(
                        osb[:, qlo:qhi, h, :], acc[:, qlo:qhi, :HD],
                        r_[:, :, None].to_broadcast([128, nqc, HD]), mult,
                    )
                    tps = {}
                    for qc in range(qlo, qhi):
                        # the final call's second transpose goes to the (then
                        # dead) acc bank so the chains don't serialize on the
                        # flex slot; earlier calls must NOT touch the acc bank
                        # (a start=True there wipes the open accumulations)
                        pool, tag = ((accp, "acc") if qc > qlo and qlo >= 2
                                     else (flexp, "flex"))
                        tp = pool.tile([128, 128], BF16, tag=tag,
                                       name=f"tpL{qc}")
                        nc.tensor.transpose(tp, osb[:, qc, :, :], ident_b)
                        tps[qc] = tp
                    for qc in range(qlo, qhi):
                        if qc == qlo:
                            nc.vector.tensor_copy(oT[:, b, qc, :], tps[qc])
                        else:
                            nc.scalar.copy(oT[:, b, qc, :], tps[qc])
                    # 4 o_proj matmuls into 4 DISTINCT psum banks (scA, scB,
                    # then the flex/acc banks that held the dead transposes)
                    # so no matmul WARs on an eviction
                    pss = {}
                    osts = {}
                    for qc in range(qlo, qhi):
                        osts[qc] = stage.tile([128, D], BF16, tag="ost",
                                              name=f"osL{qc}")
                    for nh in range(2):
                        for i, qc in enumerate(range(qlo, qhi)):
                            if nh == 0:
                                ps = scp.tile([128, 3, 512], FP32, tag="sc",
                                              name=f"pL{qc}{nh}")[:, 0, :]
                            else:
                                pool, tag = ((flexp, "flex") if i == 0
                                             else (accp, "acc"))
                                if qlo < 2 and i > 0:
                                    pool, tag = scp, "sc"
                                ps = pool.tile([128, 512], FP32, tag=tag,
                                               name=f"pL{qc}{nh}")
                                if tag == "sc":
                                    ps = ps[:, :512]
                            nc.tensor.matmul(
                                ps, lhsT=oT[:, b, qc, :],
                                rhs=woT_s[:, bass.ts(nh, 512)],
                                start=True, stop=True,
                            )
                            pss[(qc, nh)] = ps
                    # evicts: the early pair leans on ScalarE (its exp stream
                    # is ending), the last pair leans on DVE
                    for i, (qc, nh) in enumerate(
                        (q, n) for n in range(2) for q in range(qlo, qhi)
                    ):
                        ev = osts[qc][:, bass.ts(nh, 512)]
                        act_side = (i != 3) if qlo < 2 else (i >= 2)
                        if act_side:
                            nc.scalar.copy(ev, pss[(qc, nh)])
                        else:
                            nc.vector.tensor_copy(ev, pss[(qc, nh)])
                    for qc in range(qlo, qhi):
                        r0 = b * T + qc * 128
                        if qlo >= 2 and qc == qhi - 1:
                            # last row block: split per half so the first
                            # half's transfer overlaps the second's eviction
                            for nh in range(2):
                                nc.sync.dma_start(
                                    out[r0 : r0 + 128, bass.ts(nh, 512)],
                                    osts[qc][:, bass.ts(nh, 512)],
                                )
                        else:
                            nc.sync.dma_start(out[r0 : r0 + 128, :], osts[qc])

                def pv(gi):
                    pT, bases = pts[gi]
                    for cc in groups[gi]:
                        base, off = bases[cc]
                        for qc in range(max(0, cc - PCH), QC):
                            c0 = base + qc * 128 - off
                            # one start/stop per psum bank per pair: start
                            # marks the WHOLE 2KB bank pending-zero, so each
                            # region's first write overwrites (self-zeroing)
                            nc.tensor.matmul(
                                acc[:, qc, :],
                                lhsT=pT[:, c0 : c0 + 128],
                                rhs=va[:, h, cc, :],
                                start=(cc == 0 and qc == 0),
                                stop=(cc == NCH - 1 and qc == QC - 1),
                                skip_group_check=True,
                            )

                npre = len(pre) if pre is not None else 0
                for gi in range(len(groups)):
                    if gi < npre:
                        pts[gi] = pre[gi]
                    else:
                        pts[gi] = emit_group(b, h, gi)
                    if gi == len(groups) - 1 and nxt is not None:
                        nxt_pre = (emit_group(nxt[0], nxt[1], 0),)
                    if gi > 0:
                        pv(gi - 1)
                    if ci < len(chores) and gi >= npre:
                        chores[ci]()
                        ci += 1
                    if last and gi == len(groups) - 1:
                        # qc 0/1 stopped at chunks 28/29 (group 9): finish
                        # them while ScalarE still runs the last exps
                        finish_qcs(b, h, osb, 0, 2)
                pv(len(groups) - 1)
                if ci < len(chores):
                    chores[ci]()
                    ci += 1

                if last:
                    finish_qcs(b, h, osb, 2, 4)
                else:
                    # divide by the softmax denominator (column 64 of acc)
                    r_ = divp.tile([128, QC], FP32, tag="r", name=f"r{b}{h}")
                    nc.vector.reciprocal(r_, acc[:, :, HD])
                    nc.vector.tensor_tensor(
                        osb[:, :, h, :], acc[:, :, :HD],
                        r_[:, :, None].to_broadcast([128, QC, HD]), mult,
                    )
                return ci, nxt_pre

            def finish_batch(b, osb):
                # o [q, ch] -> oT [ch, q] per query chunk (both heads at once)
                tp = accp.tile([128, QC, 128], BF16, tag="acc", name=f"tp{b}")
                for qc in range(QC):
                    nc.tensor.transpose(tp[:, qc, :], osb[:, qc, :, :], ident_b)
                nc.vector.tensor_copy(oT[:, b], tp)

            # ---- prologue: only batch-0 q projection (ScalarE idle) ----
            ps_q = flexp.tile([128, 512], FP32, tag="flex", name="psq0")
            for ko in range(KO):
                nc.tensor.matmul(
                    ps_q, lhsT=w_s["q"][:, ko, :], rhs=xts[0][:, ko, :],
                    start=(ko == 0), stop=(ko == KO - 1), skip_group_check=True,
                )
            # evict on the (idle) Scalar engine: DVE would serialize this
            # behind the mask copies
            nc.scalar.copy(qT[:, :512], ps_q)
            for r in range(4):
                m = const.tile([128, T], FP32, tag=f"mask{r}", name=f"m{r}")
                nc.gpsimd.memset(m, 0.0)
                # keep 0 where query t >= key-token (128r + p), else NEG
                nc.gpsimd.affine_select(
                    out=m, in_=m, compare_op=mybir.AluOpType.is_ge,
                    fill=NEG, base=-128 * r, channel_multiplier=-1,
                    pattern=[[1, T]],
                )
                mb_ = const.tile([128, T], BF16, tag=f"maskb{r}", name=f"mb{r}")
                nc.vector.tensor_copy(mb_, m)
                masks.append(mb_)

            # ---- main loop ----
            pre = None
            for b in range(B):
                if b + 1 < B:
                    load_kv(b + 1)
                chores = []
                if b == 0:
                    chores += chores_qk(0, "k") + chores_v(0)
                else:
                    chores += chores_oproj(b - 1)
                if b + 1 < B:
                    chores += chores_qk(b + 1, "q") + chores_qk(b + 1, "k")
                    chores += chores_v(b + 1)
                osb = osbp.tile([128, QC, HPC, HD], BF16, tag="osb", name=f"osb{b}")
                ci, pre = pair(b, 0, osb, chores, 0, pre=pre, nxt=(b, 1))
                if b + 2 < B:
                    load_x(b + 2)
                nxt = (b + 1, 0) if b + 1 < B else None
                ci, pre = pair(
                    b, 1, osb, chores, ci, last=(b == B - 1), pre=pre, nxt=nxt
                )
                assert ci >= len(chores), (b, ci, len(chores))
                if b < B - 1:
                    finish_batch(b, osb)

    nc.compile()
    return nc


def _pack_v(v):
    """[B, HPC, PAST, HD] -> [B, 128, HPC, PCH, HD+1] bf16, ones in col HD."""
    o = np.empty((B, 128, HPC, PCH, HD + 1), np.float32)
    o[..., :HD] = v.reshape(B, HPC, PCH, 128, HD).transpose(0, 3, 1, 2, 4)
    o[..., HD] = 1.0
    return np.ascontiguousarray(o.astype(BF))


def _prep(x, k_prev, v_prev, Wq, Wk, Wv, Wo):
    """Host-side shard + layout marshalling (bf16, C-contiguous)."""
    f = np.float32
    x2 = np.asarray(x, f).reshape(TOK, D)
    xT = np.ascontiguousarray(x2.T.astype(BF))
    k_prev = np.asarray(k_prev, f)
    v_prev = np.asarray(v_prev, f)
    Wq, Wk, Wv, Wo = (np.asarray(w, f) for w in (Wq, Wk, Wv, Wo))

    def swz(wT):
        # [D, 128] -> [p, ko, m] with row (ko*128+p) -> [p, ko, :]
        return np.ascontiguousarray(
            wT.reshape(KO, 128, 128).transpose(1, 0, 2).astype(BF)
        )

    in_maps = []
    for c in range(NCORES):
        rows = slice(128 * c, 128 * (c + 1))
        hsl = slice(HPC * c, HPC * (c + 1))
        in_maps.append(
            {
                "xT": xT,
                "wq": swz((Wq[rows, :] * SCALE).T),
                "wk": swz(Wk[rows, :].T),
                "wv": swz(Wv[rows, :].T),
                "woT": np.ascontiguousarray(Wo[:, rows].T.astype(BF)),
                "kTp": np.ascontiguousarray(
                    k_prev[:, hsl, :, :].transpose(0, 1, 3, 2).astype(BF)
                ).reshape(B, 128, PAST),
                "vp": _pack_v(v_prev[:, hsl, :, :]),
            }
        )
    return in_maps


def kernel(x, k_prev, v_prev, Wq, Wk, Wv, Wo):
    if "nc" not in _cache:
        _cache["nc"] = _build()
    nc = _cache["nc"]
    in_maps = _prep(x, k_prev, v_prev, Wq, Wk, Wv, Wo)
    res = run_bass_kernel_spmd(nc, in_maps, core_ids=list(range(NCORES)))
    acc = np.zeros((TOK, D), np.float64)
    for r in res.results:
        acc += np.asarray(r["out"], np.float32)
    return acc.astype(np.float32).reshape(B, T, D)
